# revision 2
# baseline (speedup 1.0000x reference)
"""Trainium2 Bass kernel v2 for nn_MatchNet (MLP forward + 60-iter batched PDHG).

Data-parallel over 8 NeuronCores: batch 2048 -> 256 rows/core (2 b-tiles of 128).

v2 reformulation vs baseline: state E2 := -2*alpha*(x - Z + tau) (fp16, N layout)
plus its transpose E2T kept as a state (2 rotating buffers each). The xbar
combination aeb = E2_k - 0.5*E2_{k-1} + atau is never materialized:
  - its transpose enters ps1 via two weight sets (STa = -S^T, STb = 0.5*S^T),
    with the atau*S@1 constant folded into cSZB.
  - the y2-chain uses hA = -0.5*E2_{k-1} + E2_k (one DVE STT op).
This removes the baseline's per-iteration DVE affine (658ns) + ns/s2p ops;
s2p = relu(2 - 2*nr) is computed on Act directly ([128,1] ops are ~free).

Per iteration (tile b):
  w   = q + naZ'         (Pool STT; naZ' = -alpha*Z + atau)
  hA  = -0.5*E2p + E2c   (DVE STT)
  h   = hA + w           (DVE TT)
  qn  = relu(h)          (DVE TSP)
  ps1 = I64@pc + sum_c STb_c@E2Tp_c + sum_c STa_c@E2Tc_c   (PE, all iter-start ready)
  p   = relu(ps1)        (cfg engine)
  pcn = p + cSZB'        (cfg engine)
  NS3 = hI@E2c + nAI@qn + p@AS16          (PE PSUM [128,512] = -alpha*d)
  n2  = sum(NS3^2)       (Act Square, out->PSUM junk, accum_out)
  nr  = rsqrt(n2/t2a2+eps) (Act)
  s2p = relu(2-2*nr)     (Act)
  E2n = s2p*NS3 - 2atau  (Act h1 + DVE h2, halves)
  psT = transpose(E2n)   (PE, per half)
  E2Tn = copy(psT)       (DVE, per half)
  out x = Z - E2/(2 alpha) - tau  (from final NS3: x = Z + (s2p/(-2 alpha))*NS3)
"""

import numpy as np

N_STRUCTS = 512
N_COMBOS = 64
HID = 1024
N_ITERS = 60
N_CORES = 8
B_FULL = 2048
BC = B_FULL // N_CORES  # 256 batch rows per core
NB = BC // 128  # 2 batch sub-tiles
NF = N_STRUCTS // 128  # 4 feature chunks

OPLOG = {}

CFG = {
    "p": "dve",        # p = relu(ps1): act | dve
    "pc": "pool",      # pcn engine (unused when pcnelim)
    "w": "pool",       # w = q + naZ'
    "e2n": "act",      # E2n: act | split
    "halves": False,
    "koff": 6,
    "pcnelim": True,   # fold pc = p + cSZB into two ps1 matmuls
    "ehroute": False,  # hA-STT route for the y2 chain
    "phprio": 24,      # scheduler priority boost for p
}

_BUILD_CACHE = {}


def _power_L(S: np.ndarray) -> float:
    S = S.astype(np.float32)
    n = S.shape[1]
    v = np.full((n,), 1.0 / np.sqrt(n), np.float32)
    for _ in range(30):
        v2 = (S.T @ (S @ v) + v).astype(np.float32)
        v = (v2 / np.float32(np.linalg.norm(v2))).astype(np.float32)
    L = np.sqrt(np.vdot(v, (S.T @ (S @ v) + v).astype(np.float32)))
    return float(L)


def _build_nc(tau: float, sigma: float):
    import contextlib

    import concourse.bacc as bacc
    import concourse.mybir as mybir
    import concourse.tile as tile

    f32 = mybir.dt.float32
    f32r = mybir.dt.float32r
    f16 = mybir.dt.float16
    AF = mybir.ActivationFunctionType
    ALU = mybir.AluOpType
    alpha = float(np.float32(tau) * np.float32(sigma))
    atau = float(np.float32(alpha) * np.float32(tau))
    t2a2 = float((np.float32(tau) * np.float32(alpha)) ** 2)
    dsq_scale = float(1.0 / t2a2)

    nc = bacc.Bacc("TRN2", target_bir_lowering=False, debug=False)

    def L(label, inst):
        try:
            OPLOG[str(inst.ins.name)] = label
        except Exception:
            pass
        return inst

    def creg(v):
        key = (f32, v)
        if key not in nc.const_aps.aps:
            t = nc.alloc_sbuf_tensor(f"constx-{v}", [128, 1], f32)
            nc.gpsimd.memset(t.ap(), v)
            nc.const_aps.aps[key] = t.ap()
        return v

    creg(atau)
    creg(-2.0 * atau)
    creg(2.0)
    creg(0.0)
    creg(1e-6)

    # ---- DRAM I/O (per-core shapes) ----
    d_XT = nc.dram_tensor("xt", [N_COMBOS, BC], f32r, kind="ExternalInput")
    d_W1 = nc.dram_tensor("w1", [N_COMBOS, HID], f32r, kind="ExternalInput")
    d_b1 = nc.dram_tensor("b1r", [128, 8], f32, kind="ExternalInput")
    d_W2 = nc.dram_tensor("w2", [HID, HID], f16, kind="ExternalInput")
    d_b2 = nc.dram_tensor("b2r", [128, 8], f32, kind="ExternalInput")
    d_W3 = nc.dram_tensor("w3", [HID, N_STRUCTS], f16, kind="ExternalInput")
    d_b3 = nc.dram_tensor("b3r", [128, 4], f32, kind="ExternalInput")
    d_aST = nc.dram_tensor("ast", [128, NF * N_COMBOS], f32r, kind="ExternalInput")
    d_STa = nc.dram_tensor("sta16", [128, NF * N_COMBOS], f16, kind="ExternalInput")
    d_STb = nc.dram_tensor("stb16", [128, NF * N_COMBOS], f16, kind="ExternalInput")
    d_AS16 = nc.dram_tensor("as16", [N_COMBOS, N_STRUCTS], f16, kind="ExternalInput")
    d_nAI16 = nc.dram_tensor("nai16", [128, 128], f16, kind="ExternalInput")
    d_hI16 = nc.dram_tensor("hi16", [128, 128], f16, kind="ExternalInput")
    d_I64 = nc.dram_tensor("i64_16", [N_COMBOS, N_COMBOS], f16, kind="ExternalInput")
    d_I16 = nc.dram_tensor("i16", [128, 128], f16, kind="ExternalInput")
    d_Ir = nc.dram_tensor("identr", [128, 128], f32r, kind="ExternalInput")
    d_sb = nc.dram_tensor("sbias", [N_COMBOS, 1], f32, kind="ExternalInput")
    d_out = nc.dram_tensor("out", [BC, N_STRUCTS], f32, kind="ExternalOutput")

    FW = N_STRUCTS  # 512

    with tile.TileContext(nc) as tc:
        stack = contextlib.ExitStack()
        with stack:
            cpool = stack.enter_context(tc.tile_pool(name="consts", bufs=1))

            def cload(dram, shape, tag, dt):
                t = cpool.tile(shape, dt, tag=tag, name=tag)
                nc.sync.dma_start(t[:], dram.ap())
                return t

            XT = cload(d_XT, [N_COMBOS, BC], "xt", f32r)
            W1 = cload(d_W1, [N_COMBOS, HID], "w1", f32r)
            b1r = cload(d_b1, [128, 8], "b1r", f32)

            # ---- MLP forward (float32r, T layout) ----
            zt = []  # Z^T tiles [128, BC] x4, f32r
            with (
                tc.tile_pool(name="mlp_sb", bufs=1) as mpool,
                tc.tile_pool(name="mlp_ps", bufs=1, space="PSUM") as mpsum,
            ):
                W2 = []
                for k in range(8):
                    t = mpool.tile([128, HID], f16, tag=f"w2_{k}", name=f"w2_{k}")
                    nc.sync.dma_start(t[:], d_W2.ap()[k * 128 : (k + 1) * 128, :])
                    W2.append(t)
                b2r = cload(d_b2, [128, 8], "b2r", f32)
                W3 = []
                for k in range(8):
                    t = mpool.tile([128, N_STRUCTS], f16, tag=f"w3_{k}", name=f"w3_{k}")
                    nc.sync.dma_start(t[:], d_W3.ap()[k * 128 : (k + 1) * 128, :])
                    W3.append(t)
                b3r = cload(d_b3, [128, 4], "b3r", f32)
                aST = cload(d_aST, [128, NF * N_COMBOS], "ast", f32r)
                Ir = cload(d_Ir, [128, 128], "identr", f32r)
                STa = cload(d_STa, [128, NF * N_COMBOS], "sta16", f16)
                STb = cload(d_STb, [128, NF * N_COMBOS], "stb16", f16)
                AS16 = cload(d_AS16, [N_COMBOS, N_STRUCTS], "as16", f16)
                nAI16 = cload(d_nAI16, [128, 128], "nai16", f16)
                hI16 = cload(d_hI16, [128, 128], "hi16", f16)
                I64 = cload(d_I64, [N_COMBOS, N_COMBOS], "i64_16", f16)
                I16 = cload(d_I16, [128, 128], "i16", f16)
                sbias = cload(d_sb, [N_COMBOS, 1], "sbias", f32)
                z1t = []
                for t in range(8):
                    ps = mpsum.tile([128, BC], f32, tag=f"zmm{t}", name=f"zmm{t}")
                    nc.tensor.matmul(
                        ps[:], W1[:, t * 128 : (t + 1) * 128], XT[:], start=True, stop=True
                    )
                    sb = mpool.tile([128, BC], f16, tag=f"z1_{t}", name=f"z1_{t}")
                    nc.scalar.activation(sb[:], ps[:], AF.Relu, bias=b1r[:, t : t + 1])
                    z1t.append(sb)
                zps2 = [
                    mpsum.tile([128, BC], f32, tag=f"zmm{t}", name=f"zmm2{t}")
                    for t in range(8)
                ]
                for k in range(8):
                    for t in range(8):
                        nc.tensor.matmul(
                            zps2[t][:],
                            W2[k][:, t * 128 : (t + 1) * 128],
                            z1t[k][:],
                            start=(k == 0),
                            stop=(k == 7),
                        )
                z2t = []
                for t in range(8):
                    sb = mpool.tile([128, BC], f16, tag=f"z2_{t}", name=f"z2_{t}")
                    nc.scalar.activation(sb[:], zps2[t][:], AF.Relu, bias=b2r[:, t : t + 1])
                    z2t.append(sb)
                zps3 = [
                    mpsum.tile([128, BC], f32, tag=f"zmm{c}", name=f"z3mm{c}")
                    for c in range(NF)
                ]
                for k in range(8):
                    for c in range(NF):
                        nc.tensor.matmul(
                            zps3[c][:],
                            W3[k][:, c * 128 : (c + 1) * 128],
                            z2t[k][:],
                            start=(k == 0),
                            stop=(k == 7),
                        )
                for c in range(NF):
                    sb = cpool.tile([128, BC], f32r, tag=f"zt_{c}", name=f"zt_{c}")
                    nc.scalar.activation(sb[:], zps3[c][:], AF.Relu, bias=b3r[:, c : c + 1])
                    zt.append(sb)

            # ---- PDHG setup ----
            spool = stack.enter_context(tc.tile_pool(name="setup", bufs=1))
            e2_pool = stack.enter_context(tc.tile_pool(name="e2p", bufs=3))
            e2t_pool = stack.enter_context(tc.tile_pool(name="e2tp", bufs=3))
            q_pool = stack.enter_context(tc.tile_pool(name="qp", bufs=3))
            p_pool = stack.enter_context(tc.tile_pool(name="pp", bufs=3))
            sc_pool = stack.enter_context(tc.tile_pool(name="scratch", bufs=3))
            with tc.tile_pool(name="pd_ps", bufs=1, space="PSUM") as ppool:
                # cSZB' = alpha*(S@Z^T - B^T) - atau*(S@1) 1^T   [64, BC] fp16
                ps = ppool.tile([N_COMBOS, BC], f32, tag="py1", name="py1")
                for c in range(NF):
                    nc.tensor.matmul(
                        ps[:], aST[:, c * 64 : (c + 1) * 64], zt[c][:],
                        start=(c == 0), stop=False,
                    )
                naI64 = spool.tile([N_COMBOS, N_COMBOS], f32r, tag="nai64", name="naI64")
                nc.scalar.activation(naI64[:], Ir[:64, :64].bitcast(f32), AF.Copy, scale=-alpha)
                nc.tensor.matmul(ps[:], naI64[:], XT[:], start=False, stop=True)
                cSZB0 = spool.tile([N_COMBOS, BC], f16, tag="cszb0", name="cSZB0")
                nc.scalar.activation(cSZB0[:], ps[:], AF.Copy)
                # cSZB' = cSZB + sbias (per-partition const, folds atau*S@1)
                cSZB = spool.tile([N_COMBOS, BC], f16, tag="cszb", name="cSZB")
                nc.vector.tensor_scalar(
                    cSZB[:], cSZB0[:], sbias[:], 0.0, op0=ALU.add, op1=ALU.add
                )

                # Z per-b in N layout (f32) via PE transposes
                Zf = []
                for b in range(NB):
                    psz = ppool.tile([128, FW], f32r, tag=f"pz{b}", name=f"pz{b}")
                    for c in range(NF):
                        nc.tensor.transpose(
                            psz[:, c * 128 : (c + 1) * 128],
                            zt[c][:, b * 128 : (b + 1) * 128],
                            Ir[:],
                        )
                    zb = spool.tile([128, FW], f32, tag=f"zn{b}", name=f"zn{b}")
                    nc.scalar.activation(zb[:], psz[:].bitcast(f32), AF.Copy)
                    Zf.append(zb)

                # state init
                naZ, E2, E2p, E2T, E2Tp, q, pc = [], [], [], [], [], [], []
                for b in range(NB):
                    t = spool.tile([128, FW], f16, tag=f"naz{b}", name=f"naz{b}")
                    nc.vector.tensor_scalar(t[:], Zf[b][:], -alpha, atau, op0=ALU.mult, op1=ALU.add)
                    naZ.append(t)
                    t = e2_pool.tile([128, FW], f16, tag=f"e2{b}", name=f"e2i{b}")
                    nc.vector.tensor_scalar(t[:], Zf[b][:], 2.0 * alpha, -2.0 * atau, op0=ALU.mult, op1=ALU.add)
                    E2.append(t)
                    E2p.append(t)  # E2_{-1} = E2_0
                    # E2T_0 via PE transpose + evac
                    pst = ppool.tile([128, FW], f16, tag=f"pTi{b}", name=f"pTi{b}")
                    for c in range(NF):
                        nc.tensor.transpose(
                            pst[:, c * 128 : (c + 1) * 128],
                            t[:, c * 128 : (c + 1) * 128],
                            I16[:],
                        )
                    tt_ = e2t_pool.tile([128, FW], f16, tag=f"e2t{b}", name=f"e2ti{b}")
                    nc.vector.tensor_copy(tt_[:], pst[:])
                    E2T.append(tt_)
                    E2Tp.append(tt_)
                    tq = q_pool.tile([128, FW], f16, tag=f"q{b}", name=f"qi{b}")
                    nc.gpsimd.memset(tq[:], 0.0)
                    q.append(tq)
                    tp_ = p_pool.tile([N_COMBOS, 128], f16, tag=f"pc{b}", name=f"pci{b}")
                    nc.vector.tensor_copy(tp_[:], cSZB[:, b * 128 : (b + 1) * 128])
                    pc.append(tp_)

            ps_T = stack.enter_context(tc.tile_pool(name="ps_T", bufs=1, space="PSUM"))
            ps_y1 = stack.enter_context(tc.tile_pool(name="ps_y1", bufs=1, space="PSUM"))
            ps_3 = stack.enter_context(tc.tile_pool(name="ps_3", bufs=1, space="PSUM"))
            ps_j = stack.enter_context(tc.tile_pool(name="ps_j", bufs=1, space="PSUM"))

            # ---- iteration emission (v1 shape: trans+evac right after E2n) ----
            NSTG = 12
            K_OFF = CFG.get("koff", 6)
            temps = [dict(), dict()]
            HALF = FW // 2
            PCN_ELIM = CFG.get("pcnelim", False)
            EH_ROUTE = CFG.get("ehroute", False)

            # Eh_0 = -0.5*E2_0 ; q-chain init: h_0 = 0.5*E2_0 + naZ, qn_0 = relu(h_0)
            Eh = []
            for b in range(NB):
                t = sc_pool.tile([128, FW], f16, tag=f"eh{b}", name=f"ehi{b}")
                nc.vector.tensor_scalar(t[:], E2[b][:], -0.5, 0.0, op0=ALU.mult, op1=ALU.add)
                Eh.append(t)
            qn_next = []
            for b in range(NB):
                h0 = sc_pool.tile([128, FW], f16, tag=f"h{b}", name=f"h0{b}")
                nc.vector.scalar_tensor_tensor(
                    h0[:], E2[b][:], 0.5, naZ[b][:], op0=ALU.mult, op1=ALU.add
                )
                t = q_pool.tile([128, FW], f16, tag=f"q{b}", name=f"qn0{b}")
                nc.vector.tensor_scalar_max(t[:], h0[:], 0.0)
                qn_next.append(t)
            qn = [None, None]
            p_state = []
            for b in range(NB):
                t = p_pool.tile([N_COMBOS, 128], f16, tag=f"p{b}", name=f"pz{b}")
                nc.gpsimd.memset(t[:], 0.0)
                p_state.append(t)

            def emit(it, b, s):
                T = temps[b]
                last = it == N_ITERS - 1
                if s == 0:
                    qn[b] = qn_next[b]
                    # w_{k+1} = qn_k + naZ (Pool) -- for NEXT iter's h
                    if not last:
                        T["w"] = sc_pool.tile([128, FW], f16, tag=f"w{b}", name=f"w{b}")
                        if CFG["w"] == "pool":
                            L(f"w.{b}", nc.gpsimd.tensor_tensor(
                                T["w"][:], qn[b][:], naZ[b][:], ALU.add
                            ))
                        else:
                            L(f"w.{b}", nc.vector.tensor_tensor(T["w"][:], qn[b][:], naZ[b][:], ALU.add))
                elif s == 1:
                    pass
                elif s == 2:
                    pass
                elif s == 3:
                    pass
                elif s == 4:
                    T["ps1"] = ps_y1.tile([N_COMBOS, 128], f32, tag=f"py{b}", name=f"py{b}")
                    if PCN_ELIM:
                        L(f"ps1c.{b}", nc.tensor.matmul(
                            T["ps1"][:], I64[:], cSZB[:, b * 128 : (b + 1) * 128],
                            start=True, stop=False,
                        ))
                        L(f"ps1p.{b}", nc.tensor.matmul(
                            T["ps1"][:], I64[:], p_state[b][:], start=False, stop=False,
                        ))
                    else:
                        L(f"ps1c.{b}", nc.tensor.matmul(
                            T["ps1"][:], I64[:], pc[b][:], start=True, stop=False,
                        ))
                    for c in range(NF):
                        L(f"ps1b{c}.{b}", nc.tensor.matmul(
                            T["ps1"][:],
                            STb[:, c * 64 : (c + 1) * 64],
                            E2Tp[b][:, c * 128 : (c + 1) * 128],
                            start=False, stop=False,
                        ))
                    for c in range(NF):
                        L(f"ps1a{c}.{b}", nc.tensor.matmul(
                            T["ps1"][:],
                            STa[:, c * 64 : (c + 1) * 64],
                            E2T[b][:, c * 128 : (c + 1) * 128],
                            start=False, stop=(c == NF - 1),
                        ))
                elif s == 5:
                    T["p"] = p_pool.tile([N_COMBOS, 128], f16, tag=f"p{b}", name=f"p{b}")
                    hp = tc.high_priority(CFG.get("phprio")) if CFG.get("phprio") else contextlib.nullcontext()
                    with hp:
                        if CFG["p"] == "act":
                            L(f"p.{b}", nc.scalar.activation(T["p"][:], T["ps1"][:], AF.Relu))
                        else:
                            L(f"p.{b}", nc.vector.tensor_scalar_max(T["p"][:], T["ps1"][:], 0.0))
                elif s == 6:
                    if not PCN_ELIM and not last:
                        T["pcn"] = p_pool.tile([N_COMBOS, 128], f16, tag=f"pc{b}", name=f"pc{b}")
                        if CFG["pc"] == "pool":
                            L(f"pcn.{b}", nc.gpsimd.tensor_tensor(
                                T["pcn"][:], T["p"][:], cSZB[:, b * 128 : (b + 1) * 128], ALU.add
                            ))
                        else:
                            L(f"pcn.{b}", nc.vector.tensor_tensor(
                                T["pcn"][:], T["p"][:], cSZB[:, b * 128 : (b + 1) * 128], ALU.add
                            ))
                elif s == 7:
                    T["ns3"] = ps_3.tile([128, FW], f32, tag=f"p3{b}", name=f"p3{b}")
                    L(f"mm1.{b}", nc.tensor.matmul(T["ns3"][:], hI16[:], E2[b][:], start=True, stop=False))
                    L(f"mm2.{b}", nc.tensor.matmul(T["ns3"][:], nAI16[:], qn[b][:], start=False, stop=False))
                    L(f"mmC.{b}", nc.tensor.matmul(T["ns3"][:], T["p"][:], AS16[:], start=False, stop=True))
                elif s == 8:
                    T["n2"] = sc_pool.tile([128, 1], f32, tag=f"n2{b}", name=f"n2{b}")
                    dsqj = ps_j.tile([128, FW], f32, tag="dsqj", name=f"dsqj{b}")
                    L(f"dsq.{b}", nc.scalar.activation(dsqj[:], T["ns3"][:], AF.Square, accum_out=T["n2"][:]))
                elif s == 9:
                    T["nr"] = sc_pool.tile([128, 1], f32, tag=f"nr{b}", name=f"nr{b}")
                    L(f"nr.{b}", nc.scalar.activation(
                        T["nr"][:], T["n2"][:], AF.Abs_reciprocal_sqrt, scale=dsq_scale, bias=1e-6
                    ))
                    T["s2p"] = sc_pool.tile([128, 1], f32, tag=f"s2p{b}", name=f"s2p{b}")
                    L(f"s2p.{b}", nc.scalar.activation(T["s2p"][:], T["nr"][:], AF.Relu, scale=-2.0, bias=2.0))
                elif s == 10:
                    if last:
                        nsa = sc_pool.tile([128, 1], f32, tag=f"nsa{b}", name=f"nsa{b}")
                        nc.vector.tensor_scalar(
                            nsa[:], T["s2p"][:], -0.5 / alpha, 0.0, op0=ALU.mult, op1=ALU.add
                        )
                        xout = sc_pool.tile([128, FW], f32, tag=f"xo{b}", name=f"xo{b}")
                        nc.vector.affine_then_add(
                            xout[:], T["ns3"][:], Zf[b][:], scale=nsa[:], bias=0.0
                        )
                        nc.sync.dma_start(d_out.ap()[b * 128 : (b + 1) * 128, :], xout[:])
                        return
                    T["E2n"] = e2_pool.tile([128, FW], f16, tag=f"e2{b}", name=f"e2n{b}")
                    if CFG["e2n"] == "split":
                        L(f"E2nA.{b}", nc.scalar.activation(
                            T["E2n"][:, 0:HALF], T["ns3"][:, 0:HALF], AF.Copy,
                            scale=T["s2p"][:], bias=-2.0 * atau,
                        ))
                        L(f"E2nB.{b}", nc.vector.tensor_scalar(
                            T["E2n"][:, HALF:FW], T["ns3"][:, HALF:FW], T["s2p"][:],
                            -2.0 * atau, op0=ALU.mult, op1=ALU.add,
                        ))
                    else:
                        L(f"E2n.{b}", nc.scalar.activation(
                            T["E2n"][:], T["ns3"][:], AF.Copy,
                            scale=T["s2p"][:], bias=-2.0 * atau,
                        ))
                    T["psT"] = ps_T.tile([128, FW], f16, tag=f"pT{b}", name=f"pT{b}")
                    for c in range(NF):
                        L(f"tr{c}.{b}", nc.tensor.transpose(
                            T["psT"][:, c * 128 : (c + 1) * 128],
                            T["E2n"][:, c * 128 : (c + 1) * 128],
                            I16[:],
                        ))
                    T["E2Tn"] = e2t_pool.tile([128, FW], f16, tag=f"e2t{b}", name=f"e2tn{b}")
                    hpe = tc.high_priority(CFG.get("ehprio")) if CFG.get("ehprio") else contextlib.nullcontext()
                    with hpe:
                        L(f"evac.{b}", nc.vector.tensor_copy(T["E2Tn"][:], T["psT"][:]))
                    # q-chain for next iter
                    if EH_ROUTE:
                        T["h"] = sc_pool.tile([128, FW], f16, tag=f"h{b}", name=f"h{b}")
                        L(f"hv.{b}", nc.vector.tensor_tensor(T["h"][:], T["w"][:], Eh[b][:], ALU.add))
                        T["h2"] = sc_pool.tile([128, FW], f16, tag=f"h2{b}", name=f"h2{b}")
                        L(f"h.{b}", nc.vector.tensor_tensor(T["h2"][:], T["h"][:], T["E2n"][:], ALU.add))
                        qn_next[b] = q_pool.tile([128, FW], f16, tag=f"q{b}", name=f"qn{b}")
                        L(f"qn.{b}", nc.vector.tensor_scalar_max(qn_next[b][:], T["h2"][:], 0.0))
                        T["Ehn"] = sc_pool.tile([128, FW], f16, tag=f"eh{b}", name=f"ehn{b}")
                        L(f"Eh.{b}", nc.vector.tensor_scalar(
                            T["Ehn"][:], T["E2n"][:], -0.5, 0.0, op0=ALU.mult, op1=ALU.add
                        ))
                    else:
                        T["hA"] = sc_pool.tile([128, FW], f16, tag=f"hA{b}", name=f"hA{b}")
                        L(f"hA.{b}", nc.vector.scalar_tensor_tensor(
                            T["hA"][:], E2[b][:], -0.5, T["E2n"][:], op0=ALU.mult, op1=ALU.add
                        ))
                        T["h"] = sc_pool.tile([128, FW], f16, tag=f"h{b}", name=f"h{b}")
                        L(f"h.{b}", nc.vector.tensor_tensor(T["h"][:], T["hA"][:], T["w"][:], ALU.add))
                        qn_next[b] = q_pool.tile([128, FW], f16, tag=f"q{b}", name=f"qn{b}")
                        L(f"qn.{b}", nc.vector.tensor_scalar_max(qn_next[b][:], T["h"][:], 0.0))
                elif s == 11:
                    if not last:
                        E2p[b] = E2[b]
                        E2[b] = T["E2n"]
                        E2Tp[b] = E2T[b]
                        E2T[b] = T["E2Tn"]
                        p_state[b] = T["p"]
                        if not PCN_ELIM:
                            pc[b] = T["pcn"]
                        if EH_ROUTE:
                            Eh[b] = T["Ehn"]

            total = N_ITERS * NSTG
            for gs in range(total + K_OFF):
                if gs < total:
                    emit(gs // NSTG, 0, gs % NSTG)
                g1 = gs - K_OFF
                if 0 <= g1 < total:
                    emit(g1 // NSTG, 1, g1 % NSTG)

    nc.finalize()
    return nc


def _get_nc(S: np.ndarray):
    key = (hash(S.tobytes()), tuple(sorted(CFG.items())))
    if key not in _BUILD_CACHE:
        L = _power_L(S)
        tau = 0.9 / L
        sigma = 0.9 / L
        _BUILD_CACHE[key] = (_build_nc(tau, sigma), tau, sigma)
    return _BUILD_CACHE[key]


def _make_in_maps(X, W1, b1, W2, b2, W3, b3, S, tau, sigma):
    f32 = np.float32
    alpha = np.float32(tau) * np.float32(sigma)
    atau = np.float32(alpha) * np.float32(tau)
    a16 = np.float16(alpha).astype(f32)
    Xflat = np.ascontiguousarray(X.reshape(B_FULL, N_COMBOS)).astype(f32)
    S = S.astype(f32)
    aST_full = (alpha * S.T).astype(f32)  # [512, 64]
    aST = np.ascontiguousarray(
        np.concatenate([aST_full[c * 128 : (c + 1) * 128, :] for c in range(NF)], axis=1)
    )
    STa_full = (-S.T).astype(np.float16)
    STa = np.ascontiguousarray(
        np.concatenate([STa_full[c * 128 : (c + 1) * 128, :] for c in range(NF)], axis=1)
    )
    STb_full = (0.5 * S.T).astype(np.float16)
    STb = np.ascontiguousarray(
        np.concatenate([STb_full[c * 128 : (c + 1) * 128, :] for c in range(NF)], axis=1)
    )
    AS16 = np.ascontiguousarray((a16 * S).astype(np.float16))
    I128 = np.eye(128, dtype=f32)
    sbias = np.ascontiguousarray((-atau * S.sum(axis=1)).astype(f32).reshape(N_COMBOS, 1))
    shared = {
        "w1": np.ascontiguousarray(W1.astype(f32)),
        "b1r": np.ascontiguousarray(b1.reshape(8, 128).T).astype(f32),
        "w2": np.ascontiguousarray(W2.astype(np.float16)),
        "b2r": np.ascontiguousarray(b2.reshape(8, 128).T).astype(f32),
        "w3": np.ascontiguousarray(W3.astype(np.float16)),
        "b3r": np.ascontiguousarray(b3.reshape(4, 128).T).astype(f32),
        "ast": aST,
        "sta16": STa,
        "stb16": STb,
        "as16": AS16,
        "nai16": np.ascontiguousarray((-a16 * I128).astype(np.float16)),
        "hi16": np.ascontiguousarray((0.5 * I128).astype(np.float16)),
        "i64_16": np.eye(N_COMBOS, dtype=np.float16),
        "i16": I128.astype(np.float16),
        "identr": I128,
        "sbias": sbias,
    }
    in_maps = []
    for c in range(N_CORES):
        xt = np.ascontiguousarray(Xflat[c * BC : (c + 1) * BC, :].T)
        in_maps.append({**shared, "xt": xt})
    return in_maps


def kernel(X, W1, b1, W2, b2, W3, b3, S, batch_size):
    from concourse.bass_utils import run_bass_kernel_spmd

    X = np.asarray(X)
    S = np.asarray(S)
    nc, tau, sigma = _get_nc(np.ascontiguousarray(S.astype(np.float32)))
    in_maps = _make_in_maps(
        X, np.asarray(W1), np.asarray(b1), np.asarray(W2), np.asarray(b2),
        np.asarray(W3), np.asarray(b3), S, tau, sigma,
    )
    res = run_bass_kernel_spmd(nc, in_maps, core_ids=list(range(N_CORES)))
    out = np.concatenate([res.results[c]["out"] for c in range(N_CORES)], axis=0)
    return out.astype(np.float32)


# revision 4
# speedup vs baseline: 1.0076x; 1.0076x over previous
"""Trainium2 Bass kernel v2 for nn_MatchNet (MLP forward + 60-iter batched PDHG).

Data-parallel over 8 NeuronCores: batch 2048 -> 256 rows/core (2 b-tiles of 128).

v2 reformulation vs baseline: state E2 := -2*alpha*(x - Z + tau) (fp16, N layout)
plus its transpose E2T kept as a state (2 rotating buffers each). The xbar
combination aeb = E2_k - 0.5*E2_{k-1} + atau is never materialized:
  - its transpose enters ps1 via two weight sets (STa = -S^T, STb = 0.5*S^T),
    with the atau*S@1 constant folded into cSZB.
  - the y2-chain uses hA = -0.5*E2_{k-1} + E2_k (one DVE STT op).
This removes the baseline's per-iteration DVE affine (658ns) + ns/s2p ops;
s2p = relu(2 - 2*nr) is computed on Act directly ([128,1] ops are ~free).

Per iteration (tile b):
  w   = q + naZ'         (Pool STT; naZ' = -alpha*Z + atau)
  hA  = -0.5*E2p + E2c   (DVE STT)
  h   = hA + w           (DVE TT)
  qn  = relu(h)          (DVE TSP)
  ps1 = I64@pc + sum_c STb_c@E2Tp_c + sum_c STa_c@E2Tc_c   (PE, all iter-start ready)
  p   = relu(ps1)        (cfg engine)
  pcn = p + cSZB'        (cfg engine)
  NS3 = hI@E2c + nAI@qn + p@AS16          (PE PSUM [128,512] = -alpha*d)
  n2  = sum(NS3^2)       (Act Square, out->PSUM junk, accum_out)
  nr  = rsqrt(n2/t2a2+eps) (Act)
  s2p = relu(2-2*nr)     (Act)
  E2n = s2p*NS3 - 2atau  (Act h1 + DVE h2, halves)
  psT = transpose(E2n)   (PE, per half)
  E2Tn = copy(psT)       (DVE, per half)
  out x = Z - E2/(2 alpha) - tau  (from final NS3: x = Z + (s2p/(-2 alpha))*NS3)
"""

import numpy as np

N_STRUCTS = 512
N_COMBOS = 64
HID = 1024
N_ITERS = 60
N_CORES = 8
B_FULL = 2048
BC = B_FULL // N_CORES  # 256 batch rows per core
NB = BC // 128  # 2 batch sub-tiles
NF = N_STRUCTS // 128  # 4 feature chunks

OPLOG = {}

CFG = {
    "p": "dve",
    "pc": "pool",
    "w": "pool",
    "e2n": "act",
    "halves": False,
    "koff": 6,
    "pcnelim": True,
    "ehroute": False,
    "phprio": 24,
}

_BUILD_CACHE = {}


def _power_L(S: np.ndarray) -> float:
    S = S.astype(np.float32)
    n = S.shape[1]
    v = np.full((n,), 1.0 / np.sqrt(n), np.float32)
    for _ in range(30):
        v2 = (S.T @ (S @ v) + v).astype(np.float32)
        v = (v2 / np.float32(np.linalg.norm(v2))).astype(np.float32)
    L = np.sqrt(np.vdot(v, (S.T @ (S @ v) + v).astype(np.float32)))
    return float(L)


def _build_nc(tau: float, sigma: float):
    import contextlib

    import concourse.bacc as bacc
    import concourse.mybir as mybir
    import concourse.tile as tile

    f32 = mybir.dt.float32
    f32r = mybir.dt.float32r
    f16 = mybir.dt.float16
    AF = mybir.ActivationFunctionType
    ALU = mybir.AluOpType
    alpha = float(np.float32(tau) * np.float32(sigma))
    atau = float(np.float32(alpha) * np.float32(tau))
    t2a2 = float((np.float32(tau) * np.float32(alpha)) ** 2)
    dsq_scale = float(1.0 / t2a2)

    nc = bacc.Bacc("TRN2", target_bir_lowering=False, debug=False)

    def L(label, inst):
        try:
            OPLOG[str(inst.ins.name)] = label
        except Exception:
            pass
        return inst

    def creg(v):
        key = (f32, v)
        if key not in nc.const_aps.aps:
            t = nc.alloc_sbuf_tensor(f"constx-{v}", [128, 1], f32)
            nc.gpsimd.memset(t.ap(), v)
            nc.const_aps.aps[key] = t.ap()
        return v

    creg(atau)
    creg(-2.0 * atau)
    creg(2.0)
    creg(0.0)
    creg(1e-6)

    # ---- DRAM I/O (per-core shapes) ----
    d_XT = nc.dram_tensor("xt", [N_COMBOS, BC], f32r, kind="ExternalInput")
    d_W1 = nc.dram_tensor("w1", [N_COMBOS, HID], f32r, kind="ExternalInput")
    d_b1 = nc.dram_tensor("b1r", [128, 8], f32, kind="ExternalInput")
    d_W2 = nc.dram_tensor("w2", [HID, HID], f16, kind="ExternalInput")
    d_b2 = nc.dram_tensor("b2r", [128, 8], f32, kind="ExternalInput")
    d_W3 = nc.dram_tensor("w3", [HID, N_STRUCTS], f16, kind="ExternalInput")
    d_b3 = nc.dram_tensor("b3r", [128, 4], f32, kind="ExternalInput")
    d_aST = nc.dram_tensor("ast", [128, NF * N_COMBOS], f32r, kind="ExternalInput")
    d_STa = nc.dram_tensor("sta16", [128, NF * N_COMBOS], f16, kind="ExternalInput")
    d_STb = nc.dram_tensor("stb16", [128, NF * N_COMBOS], f16, kind="ExternalInput")
    d_AS16 = nc.dram_tensor("as16", [N_COMBOS, N_STRUCTS], f16, kind="ExternalInput")
    d_nAI16 = nc.dram_tensor("nai16", [128, 128], f16, kind="ExternalInput")
    d_hI16 = nc.dram_tensor("hi16", [128, 128], f16, kind="ExternalInput")
    d_I64 = nc.dram_tensor("i64_16", [N_COMBOS, N_COMBOS], f16, kind="ExternalInput")
    d_I16 = nc.dram_tensor("i16", [128, 128], f16, kind="ExternalInput")
    d_Ir = nc.dram_tensor("identr", [128, 128], f32r, kind="ExternalInput")
    d_sb = nc.dram_tensor("sbias", [N_COMBOS, 1], f32, kind="ExternalInput")
    d_out = nc.dram_tensor("out", [BC, N_STRUCTS], f32, kind="ExternalOutput")

    FW = N_STRUCTS  # 512

    with tile.TileContext(nc) as tc:
        stack = contextlib.ExitStack()
        with stack:
            cpool = stack.enter_context(tc.tile_pool(name="consts", bufs=1))

            def cload(dram, shape, tag, dt):
                t = cpool.tile(shape, dt, tag=tag, name=tag)
                nc.sync.dma_start(t[:], dram.ap())
                return t

            XT = cload(d_XT, [N_COMBOS, BC], "xt", f32r)
            W1 = cload(d_W1, [N_COMBOS, HID], "w1", f32r)
            b1r = cload(d_b1, [128, 8], "b1r", f32)

            # ---- MLP forward (float32r, T layout) ----
            zt = []  # Z^T tiles [128, BC] x4, f32r
            with (
                tc.tile_pool(name="mlp_sb", bufs=1) as mpool,
                tc.tile_pool(name="mlp_ps", bufs=1, space="PSUM") as mpsum,
            ):
                W2 = []
                for k in range(8):
                    t = mpool.tile([128, HID], f16, tag=f"w2_{k}", name=f"w2_{k}")
                    nc.sync.dma_start(t[:], d_W2.ap()[k * 128 : (k + 1) * 128, :])
                    W2.append(t)
                b2r = cload(d_b2, [128, 8], "b2r", f32)
                W3 = []
                for k in range(8):
                    t = mpool.tile([128, N_STRUCTS], f16, tag=f"w3_{k}", name=f"w3_{k}")
                    nc.sync.dma_start(t[:], d_W3.ap()[k * 128 : (k + 1) * 128, :])
                    W3.append(t)
                b3r = cload(d_b3, [128, 4], "b3r", f32)
                aST = cload(d_aST, [128, NF * N_COMBOS], "ast", f32r)
                Ir = cload(d_Ir, [128, 128], "identr", f32r)
                STa = cload(d_STa, [128, NF * N_COMBOS], "sta16", f16)
                STb = cload(d_STb, [128, NF * N_COMBOS], "stb16", f16)
                AS16 = cload(d_AS16, [N_COMBOS, N_STRUCTS], "as16", f16)
                nAI16 = cload(d_nAI16, [128, 128], "nai16", f16)
                hI16 = cload(d_hI16, [128, 128], "hi16", f16)
                I64 = cload(d_I64, [N_COMBOS, N_COMBOS], "i64_16", f16)
                I16 = cload(d_I16, [128, 128], "i16", f16)
                sbias = cload(d_sb, [N_COMBOS, 1], "sbias", f32)
                z1t = []
                for t in range(8):
                    ps = mpsum.tile([128, BC], f32, tag=f"zmm{t}", name=f"zmm{t}")
                    nc.tensor.matmul(
                        ps[:], W1[:, t * 128 : (t + 1) * 128], XT[:], start=True, stop=True
                    )
                    sb = mpool.tile([128, BC], f16, tag=f"z1_{t}", name=f"z1_{t}")
                    nc.scalar.activation(sb[:], ps[:], AF.Relu, bias=b1r[:, t : t + 1])
                    z1t.append(sb)
                zps2 = [
                    mpsum.tile([128, BC], f32, tag=f"zmm{t}", name=f"zmm2{t}")
                    for t in range(8)
                ]
                for k in range(8):
                    for t in range(8):
                        nc.tensor.matmul(
                            zps2[t][:],
                            W2[k][:, t * 128 : (t + 1) * 128],
                            z1t[k][:],
                            start=(k == 0),
                            stop=(k == 7),
                        )
                z2t = []
                for t in range(8):
                    sb = mpool.tile([128, BC], f16, tag=f"z2_{t}", name=f"z2_{t}")
                    nc.scalar.activation(sb[:], zps2[t][:], AF.Relu, bias=b2r[:, t : t + 1])
                    z2t.append(sb)
                zps3 = [
                    mpsum.tile([128, BC], f32, tag=f"zmm{c}", name=f"z3mm{c}")
                    for c in range(NF)
                ]
                for k in range(8):
                    for c in range(NF):
                        nc.tensor.matmul(
                            zps3[c][:],
                            W3[k][:, c * 128 : (c + 1) * 128],
                            z2t[k][:],
                            start=(k == 0),
                            stop=(k == 7),
                        )
                for c in range(NF):
                    sb = cpool.tile([128, BC], f32r, tag=f"zt_{c}", name=f"zt_{c}")
                    nc.scalar.activation(sb[:], zps3[c][:], AF.Relu, bias=b3r[:, c : c + 1])
                    zt.append(sb)

            # ---- PDHG setup ----
            spool = stack.enter_context(tc.tile_pool(name="setup", bufs=1))
            e2_pool = stack.enter_context(tc.tile_pool(name="e2p", bufs=3))
            e2t_pool = stack.enter_context(tc.tile_pool(name="e2tp", bufs=3))
            q_pool = stack.enter_context(tc.tile_pool(name="qp", bufs=3))
            p_pool = stack.enter_context(tc.tile_pool(name="pp", bufs=3))
            sc_pool = stack.enter_context(tc.tile_pool(name="scratch", bufs=3))
            with tc.tile_pool(name="pd_ps", bufs=1, space="PSUM") as ppool:
                # cSZB' = alpha*(S@Z^T - B^T) - atau*(S@1) 1^T   [64, BC] fp16
                ps = ppool.tile([N_COMBOS, BC], f32, tag="py1", name="py1")
                for c in range(NF):
                    nc.tensor.matmul(
                        ps[:], aST[:, c * 64 : (c + 1) * 64], zt[c][:],
                        start=(c == 0), stop=False,
                    )
                naI64 = spool.tile([N_COMBOS, N_COMBOS], f32r, tag="nai64", name="naI64")
                nc.scalar.activation(naI64[:], Ir[:64, :64].bitcast(f32), AF.Copy, scale=-alpha)
                nc.tensor.matmul(ps[:], naI64[:], XT[:], start=False, stop=True)
                cSZB0 = spool.tile([N_COMBOS, BC], f16, tag="cszb0", name="cSZB0")
                nc.scalar.activation(cSZB0[:], ps[:], AF.Copy)
                # cSZB' = cSZB + sbias (per-partition const, folds atau*S@1)
                cSZB = spool.tile([N_COMBOS, BC], f16, tag="cszb", name="cSZB")
                nc.vector.tensor_scalar(
                    cSZB[:], cSZB0[:], sbias[:], 0.0, op0=ALU.add, op1=ALU.add
                )

                # Z per-b in N layout (f32) via PE transposes
                Zf = []
                for b in range(NB):
                    psz = ppool.tile([128, FW], f32r, tag=f"pz{b}", name=f"pz{b}")
                    for c in range(NF):
                        nc.tensor.transpose(
                            psz[:, c * 128 : (c + 1) * 128],
                            zt[c][:, b * 128 : (b + 1) * 128],
                            Ir[:],
                        )
                    zb = spool.tile([128, FW], f32, tag=f"zn{b}", name=f"zn{b}")
                    nc.scalar.activation(zb[:], psz[:].bitcast(f32), AF.Copy)
                    Zf.append(zb)

                # state init
                naZ, E2, E2p, E2T, E2Tp, q, pc = [], [], [], [], [], [], []
                for b in range(NB):
                    t = spool.tile([128, FW], f16, tag=f"naz{b}", name=f"naz{b}")
                    nc.scalar.activation(t[:], Zf[b][:], AF.Copy, scale=-alpha, bias=atau)
                    naZ.append(t)
                    t = e2_pool.tile([128, FW], f16, tag=f"e2{b}", name=f"e2i{b}")
                    nc.vector.tensor_scalar(t[:], Zf[b][:], 2.0 * alpha, -2.0 * atau, op0=ALU.mult, op1=ALU.add)
                    E2.append(t)
                    E2p.append(t)  # E2_{-1} = E2_0
                    # E2T_0 via PE transpose + evac
                    pst = ppool.tile([128, FW], f16, tag=f"pTi{b}", name=f"pTi{b}")
                    for c in range(NF):
                        nc.tensor.transpose(
                            pst[:, c * 128 : (c + 1) * 128],
                            t[:, c * 128 : (c + 1) * 128],
                            I16[:],
                        )
                    tt_ = e2t_pool.tile([128, FW], f16, tag=f"e2t{b}", name=f"e2ti{b}")
                    nc.vector.tensor_copy(tt_[:], pst[:])
                    E2T.append(tt_)
                    E2Tp.append(tt_)
                    tq = q_pool.tile([128, FW], f16, tag=f"q{b}", name=f"qi{b}")
                    nc.gpsimd.memset(tq[:], 0.0)
                    q.append(tq)
                    tp_ = p_pool.tile([N_COMBOS, 128], f16, tag=f"pc{b}", name=f"pci{b}")
                    nc.vector.tensor_copy(tp_[:], cSZB[:, b * 128 : (b + 1) * 128])
                    pc.append(tp_)

            ps_T = stack.enter_context(tc.tile_pool(name="ps_T", bufs=(2 if CFG.get("sharedT") else CFG.get("bufsT", 1)), space="PSUM"))
            ps_y1 = stack.enter_context(tc.tile_pool(name="ps_y1", bufs=CFG.get("bufsY", 1), space="PSUM"))
            ps_3 = stack.enter_context(tc.tile_pool(name="ps_3", bufs=(2 if CFG.get("shared3") else 1), space="PSUM"))
            ps_j = stack.enter_context(tc.tile_pool(name="ps_j", bufs=1, space="PSUM"))

            # ---- iteration emission (v1 shape: trans+evac right after E2n) ----
            NSTG = 12
            K_OFF = CFG.get("koff", 6)
            temps = [dict(), dict()]
            HALF = FW // 2
            PCN_ELIM = CFG.get("pcnelim", False)
            EH_ROUTE = CFG.get("ehroute", False)

            # Analytic init: h_0 = w_0 + aeb_0 = (atau - aZ) + aZ ... = 0 exactly,
            # so qn_0 = 0; and Eh_0 = -0.5*E2_0 = -aZ + atau = naZ (alias).
            Eh = list(naZ)
            qn_next = []
            for b in range(NB):
                t = q_pool.tile([128, FW], f16, tag=f"q{b}", name=f"qn0{b}")
                nc.gpsimd.memset(t[:], 0.0)
                qn_next.append(t)
            qn = [None, None]
            p_state = []
            for b in range(NB):
                t = p_pool.tile([N_COMBOS, 128], f16, tag=f"p{b}", name=f"pz{b}")
                nc.gpsimd.memset(t[:], 0.0)
                p_state.append(t)

            def emit(it, b, s):
                T = temps[b]
                last = it == N_ITERS - 1
                if s == 0:
                    qn[b] = qn_next[b]
                    # w_{k+1} = qn_k + naZ (Pool) -- for NEXT iter's h
                    if not last:
                        T["w"] = sc_pool.tile([128, FW], f16, tag=f"w{b}", name=f"w{b}")
                        if CFG["w"] == "pool":
                            L(f"w.{b}", nc.gpsimd.tensor_tensor(
                                T["w"][:], qn[b][:], naZ[b][:], ALU.add
                            ))
                        else:
                            L(f"w.{b}", nc.vector.tensor_tensor(T["w"][:], qn[b][:], naZ[b][:], ALU.add))
                elif s == 1:
                    if EH_ROUTE and not last:
                        T["hv"] = sc_pool.tile([128, FW], f16, tag=f"hv{b}", name=f"hv{b}")
                        L(f"hv.{b}", nc.vector.tensor_tensor(T["hv"][:], T["w"][:], Eh[b][:], ALU.add))
                elif s == 2:
                    if CFG.get("earlyps1"):
                        T["ps1"] = ps_y1.tile([N_COMBOS, 128], f32, tag=f"py{b}", name=f"py{b}")
                        L(f"ps1c.{b}", nc.tensor.matmul(
                            T["ps1"][:], I64[:], cSZB[:, b * 128 : (b + 1) * 128],
                            start=True, stop=False,
                        ))
                        L(f"ps1p.{b}", nc.tensor.matmul(
                            T["ps1"][:], I64[:], p_state[b][:], start=False, stop=False,
                        ))
                        for c in range(NF):
                            L(f"ps1b{c}.{b}", nc.tensor.matmul(
                                T["ps1"][:],
                                STb[:, c * 64 : (c + 1) * 64],
                                E2Tp[b][:, c * 128 : (c + 1) * 128],
                                start=False, stop=False,
                            ))
                elif s == 3:
                    pass
                elif s == 4:
                    if CFG.get("earlyps1"):
                        for c in range(NF):
                            L(f"ps1a{c}.{b}", nc.tensor.matmul(
                                T["ps1"][:],
                                STa[:, c * 64 : (c + 1) * 64],
                                E2T[b][:, c * 128 : (c + 1) * 128],
                                start=False, stop=(c == NF - 1),
                            ))
                        return
                    T["ps1"] = ps_y1.tile([N_COMBOS, 128], f32, tag=f"py{b}", name=f"py{b}")
                    if PCN_ELIM:
                        L(f"ps1c.{b}", nc.tensor.matmul(
                            T["ps1"][:], I64[:], cSZB[:, b * 128 : (b + 1) * 128],
                            start=True, stop=False,
                        ))
                        L(f"ps1p.{b}", nc.tensor.matmul(
                            T["ps1"][:], I64[:], p_state[b][:], start=False, stop=False,
                        ))
                    else:
                        L(f"ps1c.{b}", nc.tensor.matmul(
                            T["ps1"][:], I64[:], pc[b][:], start=True, stop=False,
                        ))
                    for c in range(NF):
                        L(f"ps1b{c}.{b}", nc.tensor.matmul(
                            T["ps1"][:],
                            STb[:, c * 64 : (c + 1) * 64],
                            E2Tp[b][:, c * 128 : (c + 1) * 128],
                            start=False, stop=False,
                        ))
                    for c in range(NF):
                        L(f"ps1a{c}.{b}", nc.tensor.matmul(
                            T["ps1"][:],
                            STa[:, c * 64 : (c + 1) * 64],
                            E2T[b][:, c * 128 : (c + 1) * 128],
                            start=False, stop=(c == NF - 1),
                        ))
                elif s == 5:
                    T["p"] = p_pool.tile([N_COMBOS, 128], f16, tag=f"p{b}", name=f"p{b}")
                    hp = tc.high_priority(CFG.get("phprio")) if CFG.get("phprio") else contextlib.nullcontext()
                    with hp:
                        if CFG["p"] == "act":
                            L(f"p.{b}", nc.scalar.activation(T["p"][:], T["ps1"][:], AF.Relu))
                        else:
                            L(f"p.{b}", nc.vector.tensor_scalar_max(T["p"][:], T["ps1"][:], 0.0))
                elif s == 6:
                    if not PCN_ELIM and not last:
                        T["pcn"] = p_pool.tile([N_COMBOS, 128], f16, tag=f"pc{b}", name=f"pc{b}")
                        if CFG["pc"] == "pool":
                            L(f"pcn.{b}", nc.gpsimd.tensor_tensor(
                                T["pcn"][:], T["p"][:], cSZB[:, b * 128 : (b + 1) * 128], ALU.add
                            ))
                        else:
                            L(f"pcn.{b}", nc.vector.tensor_tensor(
                                T["pcn"][:], T["p"][:], cSZB[:, b * 128 : (b + 1) * 128], ALU.add
                            ))
                elif s == 7:
                    T["ns3"] = ps_3.tile([128, FW], f32, tag=("p3" if CFG.get("shared3") else f"p3{b}"), name=f"p3{b}")
                    L(f"mm1.{b}", nc.tensor.matmul(T["ns3"][:], hI16[:], E2[b][:], start=True, stop=False))
                    L(f"mm2.{b}", nc.tensor.matmul(T["ns3"][:], nAI16[:], qn[b][:], start=False, stop=False))
                    hpc = tc.high_priority(CFG.get("chprio")) if CFG.get("chprio") else contextlib.nullcontext()
                    with hpc:
                        L(f"mmC.{b}", nc.tensor.matmul(T["ns3"][:], T["p"][:], AS16[:], start=False, stop=True))
                elif s == 8:
                    T["n2"] = sc_pool.tile([128, 1], f32, tag=f"n2{b}", name=f"n2{b}")
                    if CFG.get("jsbuf"):
                        dsqj = sc_pool.tile([128, FW], f32, tag="dsqj", name=f"dsqj{b}")
                    else:
                        dsqj = ps_j.tile([128, FW], f32, tag="dsqj", name=f"dsqj{b}")
                    L(f"dsq.{b}", nc.scalar.activation(dsqj[:], T["ns3"][:], AF.Square, accum_out=T["n2"][:]))
                elif s == 9:
                    T["nr"] = sc_pool.tile([128, 1], f32, tag=f"nr{b}", name=f"nr{b}")
                    L(f"nr.{b}", nc.scalar.activation(
                        T["nr"][:], T["n2"][:], AF.Abs_reciprocal_sqrt, scale=dsq_scale, bias=1e-6
                    ))
                    T["s2p"] = sc_pool.tile([128, 1], f32, tag=f"s2p{b}", name=f"s2p{b}")
                    L(f"s2p.{b}", nc.scalar.activation(T["s2p"][:], T["nr"][:], AF.Relu, scale=-2.0, bias=2.0))
                elif s == 10:
                    if last:
                        nsa = sc_pool.tile([128, 1], f32, tag=f"nsa{b}", name=f"nsa{b}")
                        nc.vector.tensor_scalar(
                            nsa[:], T["s2p"][:], -0.5 / alpha, 0.0, op0=ALU.mult, op1=ALU.add
                        )
                        xout = sc_pool.tile([128, FW], f32, tag=f"xo{b}", name=f"xo{b}")
                        nc.vector.affine_then_add(
                            xout[:], T["ns3"][:], Zf[b][:], scale=nsa[:], bias=0.0
                        )
                        nc.sync.dma_start(d_out.ap()[b * 128 : (b + 1) * 128, :], xout[:])
                        return
                    T["E2n"] = e2_pool.tile([128, FW], f16, tag=f"e2{b}", name=f"e2n{b}")
                    if CFG["e2n"] == "split":
                        L(f"E2nA.{b}", nc.scalar.activation(
                            T["E2n"][:, 0:HALF], T["ns3"][:, 0:HALF], AF.Copy,
                            scale=T["s2p"][:], bias=-2.0 * atau,
                        ))
                        L(f"E2nB.{b}", nc.vector.tensor_scalar(
                            T["E2n"][:, HALF:FW], T["ns3"][:, HALF:FW], T["s2p"][:],
                            -2.0 * atau, op0=ALU.mult, op1=ALU.add,
                        ))
                    else:
                        L(f"E2n.{b}", nc.scalar.activation(
                            T["E2n"][:], T["ns3"][:], AF.Copy,
                            scale=T["s2p"][:], bias=-2.0 * atau,
                        ))
                    T["psT"] = ps_T.tile([128, FW], f16, tag=("pT" if CFG.get("sharedT") else f"pT{b}"), name=f"pT{b}")
                    hpt = tc.high_priority(CFG.get("thprio")) if CFG.get("thprio") else contextlib.nullcontext()
                    with hpt:
                        for c in range(NF):
                            L(f"tr{c}.{b}", nc.tensor.transpose(
                                T["psT"][:, c * 128 : (c + 1) * 128],
                                T["E2n"][:, c * 128 : (c + 1) * 128],
                                I16[:],
                            ))
                    T["E2Tn"] = e2t_pool.tile([128, FW], f16, tag=f"e2t{b}", name=f"e2tn{b}")
                    hpe = tc.high_priority(CFG.get("ehprio")) if CFG.get("ehprio") else contextlib.nullcontext()
                    with hpe:
                        ev = CFG.get("evac", "dve")
                        if ev == "act":
                            L(f"evac.{b}", nc.scalar.activation(T["E2Tn"][:], T["psT"][:].bitcast(f16), AF.Copy))
                        elif ev == "split":
                            L(f"evacA.{b}", nc.scalar.activation(T["E2Tn"][:, 0:HALF], T["psT"][:, 0:HALF].bitcast(f16), AF.Copy))
                            L(f"evac.{b}", nc.vector.tensor_copy(T["E2Tn"][:, HALF:FW], T["psT"][:, HALF:FW]))
                        else:
                            L(f"evac.{b}", nc.vector.tensor_copy(T["E2Tn"][:], T["psT"][:]))
                    # q-chain for next iter
                    if EH_ROUTE:
                        T["h2"] = sc_pool.tile([128, FW], f16, tag=f"h2{b}", name=f"h2{b}")
                        L(f"h.{b}", nc.vector.tensor_tensor(T["h2"][:], T["hv"][:], T["E2n"][:], ALU.add))
                        qn_next[b] = q_pool.tile([128, FW], f16, tag=f"q{b}", name=f"qn{b}")
                        L(f"qn.{b}", nc.vector.tensor_scalar_max(qn_next[b][:], T["h2"][:], 0.0))
                        T["Ehn"] = sc_pool.tile([128, FW], f16, tag=f"eh{b}", name=f"ehn{b}")
                        L(f"Eh.{b}", nc.vector.tensor_scalar(
                            T["Ehn"][:], T["E2n"][:], -0.5, 0.0, op0=ALU.mult, op1=ALU.add
                        ))
                    else:
                        T["hA"] = sc_pool.tile([128, FW], f16, tag=f"hA{b}", name=f"hA{b}")
                        L(f"hA.{b}", nc.vector.scalar_tensor_tensor(
                            T["hA"][:], E2[b][:], -0.5, T["E2n"][:], op0=ALU.mult, op1=ALU.add
                        ))
                        T["h"] = sc_pool.tile([128, FW], f16, tag=f"h{b}", name=f"h{b}")
                        L(f"h.{b}", nc.vector.tensor_tensor(T["h"][:], T["hA"][:], T["w"][:], ALU.add))
                        qn_next[b] = q_pool.tile([128, FW], f16, tag=f"q{b}", name=f"qn{b}")
                        hpq = tc.high_priority(CFG.get("qhprio")) if CFG.get("qhprio") else contextlib.nullcontext()
                        with hpq:
                            L(f"qn.{b}", nc.vector.tensor_scalar_max(qn_next[b][:], T["h"][:], 0.0))
                elif s == 11:
                    if not last:
                        E2p[b] = E2[b]
                        E2[b] = T["E2n"]
                        E2Tp[b] = E2T[b]
                        E2T[b] = T["E2Tn"]
                        p_state[b] = T["p"]
                        if not PCN_ELIM:
                            pc[b] = T["pcn"]
                        if EH_ROUTE:
                            Eh[b] = T["Ehn"]

            total = N_ITERS * NSTG
            for gs in range(total + K_OFF):
                if gs < total:
                    emit(gs // NSTG, 0, gs % NSTG)
                g1 = gs - K_OFF
                if 0 <= g1 < total:
                    emit(g1 // NSTG, 1, g1 % NSTG)

    nc.finalize()
    return nc


def _get_nc(S: np.ndarray):
    key = (hash(S.tobytes()), tuple(sorted(CFG.items())))
    if key not in _BUILD_CACHE:
        L = _power_L(S)
        tau = 0.9 / L
        sigma = 0.9 / L
        _BUILD_CACHE[key] = (_build_nc(tau, sigma), tau, sigma)
    return _BUILD_CACHE[key]


def _make_in_maps(X, W1, b1, W2, b2, W3, b3, S, tau, sigma):
    f32 = np.float32
    alpha = np.float32(tau) * np.float32(sigma)
    atau = np.float32(alpha) * np.float32(tau)
    a16 = np.float16(alpha).astype(f32)
    Xflat = np.ascontiguousarray(X.reshape(B_FULL, N_COMBOS)).astype(f32)
    S = S.astype(f32)
    aST_full = (alpha * S.T).astype(f32)  # [512, 64]
    aST = np.ascontiguousarray(
        np.concatenate([aST_full[c * 128 : (c + 1) * 128, :] for c in range(NF)], axis=1)
    )
    STa_full = (-S.T).astype(np.float16)
    STa = np.ascontiguousarray(
        np.concatenate([STa_full[c * 128 : (c + 1) * 128, :] for c in range(NF)], axis=1)
    )
    STb_full = (0.5 * S.T).astype(np.float16)
    STb = np.ascontiguousarray(
        np.concatenate([STb_full[c * 128 : (c + 1) * 128, :] for c in range(NF)], axis=1)
    )
    AS16 = np.ascontiguousarray((a16 * S).astype(np.float16))
    I128 = np.eye(128, dtype=f32)
    sbias = np.ascontiguousarray((-atau * S.sum(axis=1)).astype(f32).reshape(N_COMBOS, 1))
    shared = {
        "w1": np.ascontiguousarray(W1.astype(f32)),
        "b1r": np.ascontiguousarray(b1.reshape(8, 128).T).astype(f32),
        "w2": np.ascontiguousarray(W2.astype(np.float16)),
        "b2r": np.ascontiguousarray(b2.reshape(8, 128).T).astype(f32),
        "w3": np.ascontiguousarray(W3.astype(np.float16)),
        "b3r": np.ascontiguousarray(b3.reshape(4, 128).T).astype(f32),
        "ast": aST,
        "sta16": STa,
        "stb16": STb,
        "as16": AS16,
        "nai16": np.ascontiguousarray((-a16 * I128).astype(np.float16)),
        "hi16": np.ascontiguousarray((0.5 * I128).astype(np.float16)),
        "i64_16": np.eye(N_COMBOS, dtype=np.float16),
        "i16": I128.astype(np.float16),
        "identr": I128,
        "sbias": sbias,
    }
    in_maps = []
    for c in range(N_CORES):
        xt = np.ascontiguousarray(Xflat[c * BC : (c + 1) * BC, :].T)
        in_maps.append({**shared, "xt": xt})
    return in_maps


def kernel(X, W1, b1, W2, b2, W3, b3, S, batch_size):
    from concourse.bass_utils import run_bass_kernel_spmd

    X = np.asarray(X)
    S = np.asarray(S)
    nc, tau, sigma = _get_nc(np.ascontiguousarray(S.astype(np.float32)))
    in_maps = _make_in_maps(
        X, np.asarray(W1), np.asarray(b1), np.asarray(W2), np.asarray(b2),
        np.asarray(W3), np.asarray(b3), S, tau, sigma,
    )
    res = run_bass_kernel_spmd(nc, in_maps, core_ids=list(range(N_CORES)))
    out = np.concatenate([res.results[c]["out"] for c in range(N_CORES)], axis=0)
    return out.astype(np.float32)


# revision 5
# speedup vs baseline: 1.0077x; 1.0001x over previous
"""Trainium2 Bass kernel v2 for nn_MatchNet (MLP forward + 60-iter batched PDHG).

Data-parallel over 8 NeuronCores: batch 2048 -> 256 rows/core (2 b-tiles of 128).

v2 reformulation vs baseline: state E2 := -2*alpha*(x - Z + tau) (fp16, N layout)
plus its transpose E2T kept as a state (2 rotating buffers each). The xbar
combination aeb = E2_k - 0.5*E2_{k-1} + atau is never materialized:
  - its transpose enters ps1 via two weight sets (STa = -S^T, STb = 0.5*S^T),
    with the atau*S@1 constant folded into cSZB.
  - the y2-chain uses hA = -0.5*E2_{k-1} + E2_k (one DVE STT op).
This removes the baseline's per-iteration DVE affine (658ns) + ns/s2p ops;
s2p = relu(2 - 2*nr) is computed on Act directly ([128,1] ops are ~free).

Per iteration (tile b):
  w   = q + naZ'         (Pool STT; naZ' = -alpha*Z + atau)
  hA  = -0.5*E2p + E2c   (DVE STT)
  h   = hA + w           (DVE TT)
  qn  = relu(h)          (DVE TSP)
  ps1 = I64@pc + sum_c STb_c@E2Tp_c + sum_c STa_c@E2Tc_c   (PE, all iter-start ready)
  p   = relu(ps1)        (cfg engine)
  pcn = p + cSZB'        (cfg engine)
  NS3 = hI@E2c + nAI@qn + p@AS16          (PE PSUM [128,512] = -alpha*d)
  n2  = sum(NS3^2)       (Act Square, out->PSUM junk, accum_out)
  nr  = rsqrt(n2/t2a2+eps) (Act)
  s2p = relu(2-2*nr)     (Act)
  E2n = s2p*NS3 - 2atau  (Act h1 + DVE h2, halves)
  psT = transpose(E2n)   (PE, per half)
  E2Tn = copy(psT)       (DVE, per half)
  out x = Z - E2/(2 alpha) - tau  (from final NS3: x = Z + (s2p/(-2 alpha))*NS3)
"""

import numpy as np

N_STRUCTS = 512
N_COMBOS = 64
HID = 1024
N_ITERS = 60
N_CORES = 8
B_FULL = 2048
BC = B_FULL // N_CORES  # 256 batch rows per core
NB = BC // 128  # 2 batch sub-tiles
NF = N_STRUCTS // 128  # 4 feature chunks

OPLOG = {}

CFG = {
    "p": "dve",
    "pc": "pool",
    "w": "pool",
    "e2n": "act",
    "halves": False,
    "koff": 6,
    "pcnelim": True,
    "ehroute": False,
    "phprio": 24,
}

_BUILD_CACHE = {}


def _power_L(S: np.ndarray) -> float:
    S = S.astype(np.float32)
    n = S.shape[1]
    v = np.full((n,), 1.0 / np.sqrt(n), np.float32)
    for _ in range(30):
        v2 = (S.T @ (S @ v) + v).astype(np.float32)
        v = (v2 / np.float32(np.linalg.norm(v2))).astype(np.float32)
    L = np.sqrt(np.vdot(v, (S.T @ (S @ v) + v).astype(np.float32)))
    return float(L)


def _build_nc(tau: float, sigma: float):
    import contextlib

    import concourse.bacc as bacc
    import concourse.mybir as mybir
    import concourse.tile as tile

    f32 = mybir.dt.float32
    f32r = mybir.dt.float32r
    f16 = mybir.dt.float16
    AF = mybir.ActivationFunctionType
    ALU = mybir.AluOpType
    alpha = float(np.float32(tau) * np.float32(sigma))
    atau = float(np.float32(alpha) * np.float32(tau))
    t2a2 = float((np.float32(tau) * np.float32(alpha)) ** 2)
    dsq_scale = float(1.0 / t2a2)

    nc = bacc.Bacc("TRN2", target_bir_lowering=False, debug=False)

    def L(label, inst):
        try:
            OPLOG[str(inst.ins.name)] = label
        except Exception:
            pass
        return inst

    def creg(v):
        key = (f32, v)
        if key not in nc.const_aps.aps:
            t = nc.alloc_sbuf_tensor(f"constx-{v}", [128, 1], f32)
            nc.gpsimd.memset(t.ap(), v)
            nc.const_aps.aps[key] = t.ap()
        return v

    creg(atau)
    creg(-2.0 * atau)
    creg(2.0)
    creg(0.0)
    creg(1e-6)

    # ---- DRAM I/O (per-core shapes) ----
    d_XT = nc.dram_tensor("xt", [N_COMBOS, BC], f32r, kind="ExternalInput")
    d_W1 = nc.dram_tensor("w1", [N_COMBOS, HID], f32r, kind="ExternalInput")
    d_b1 = nc.dram_tensor("b1r", [128, 8], f32, kind="ExternalInput")
    d_W2 = nc.dram_tensor("w2", [HID, HID], f16, kind="ExternalInput")
    d_b2 = nc.dram_tensor("b2r", [128, 8], f32, kind="ExternalInput")
    d_W3 = nc.dram_tensor("w3", [HID, N_STRUCTS], f16, kind="ExternalInput")
    d_b3 = nc.dram_tensor("b3r", [128, 4], f32, kind="ExternalInput")
    d_aST = nc.dram_tensor("ast", [128, NF * N_COMBOS], f32r, kind="ExternalInput")
    d_STa = nc.dram_tensor("sta16", [128, NF * N_COMBOS], f16, kind="ExternalInput")
    d_STb = nc.dram_tensor("stb16", [128, NF * N_COMBOS], f16, kind="ExternalInput")
    d_AS16 = nc.dram_tensor("as16", [N_COMBOS, N_STRUCTS], f16, kind="ExternalInput")
    d_nAI16 = nc.dram_tensor("nai16", [128, 128], f16, kind="ExternalInput")
    d_hI16 = nc.dram_tensor("hi16", [128, 128], f16, kind="ExternalInput")
    d_I64 = nc.dram_tensor("i64_16", [N_COMBOS, N_COMBOS], f16, kind="ExternalInput")
    d_I16 = nc.dram_tensor("i16", [128, 128], f16, kind="ExternalInput")
    d_Ir = nc.dram_tensor("identr", [128, 128], f32r, kind="ExternalInput")
    d_sb = nc.dram_tensor("sbias", [N_COMBOS, 1], f32, kind="ExternalInput")
    d_out = nc.dram_tensor("out", [BC, N_STRUCTS], f32, kind="ExternalOutput")

    FW = N_STRUCTS  # 512

    with tile.TileContext(nc) as tc:
        stack = contextlib.ExitStack()
        with stack:
            cpool = stack.enter_context(tc.tile_pool(name="consts", bufs=1))

            def cload(dram, shape, tag, dt):
                t = cpool.tile(shape, dt, tag=tag, name=tag)
                nc.sync.dma_start(t[:], dram.ap())
                return t

            XT = cload(d_XT, [N_COMBOS, BC], "xt", f32r)
            W1 = cload(d_W1, [N_COMBOS, HID], "w1", f32r)
            b1r = cload(d_b1, [128, 8], "b1r", f32)

            # ---- MLP forward (float32r, T layout) ----
            zt = []  # Z^T tiles [128, BC] x4, f32r
            with (
                tc.tile_pool(name="mlp_sb", bufs=1) as mpool,
                tc.tile_pool(name="mlp_ps", bufs=1, space="PSUM") as mpsum,
            ):
                W2 = []
                for k in range(8):
                    t = mpool.tile([128, HID], f16, tag=f"w2_{k}", name=f"w2_{k}")
                    nc.sync.dma_start(t[:], d_W2.ap()[k * 128 : (k + 1) * 128, :])
                    W2.append(t)
                b2r = cload(d_b2, [128, 8], "b2r", f32)
                W3 = []
                for k in range(8):
                    t = mpool.tile([128, N_STRUCTS], f16, tag=f"w3_{k}", name=f"w3_{k}")
                    nc.sync.dma_start(t[:], d_W3.ap()[k * 128 : (k + 1) * 128, :])
                    W3.append(t)
                b3r = cload(d_b3, [128, 4], "b3r", f32)
                aST = cload(d_aST, [128, NF * N_COMBOS], "ast", f32r)
                Ir = cload(d_Ir, [128, 128], "identr", f32r)
                STa = cload(d_STa, [128, NF * N_COMBOS], "sta16", f16)
                STb = cload(d_STb, [128, NF * N_COMBOS], "stb16", f16)
                AS16 = cload(d_AS16, [N_COMBOS, N_STRUCTS], "as16", f16)
                nAI16 = cload(d_nAI16, [128, 128], "nai16", f16)
                hI16 = cload(d_hI16, [128, 128], "hi16", f16)
                I64 = cload(d_I64, [N_COMBOS, N_COMBOS], "i64_16", f16)
                I16 = cload(d_I16, [128, 128], "i16", f16)
                sbias = cload(d_sb, [N_COMBOS, 1], "sbias", f32)
                z1t = []
                for t in range(8):
                    ps = mpsum.tile([128, BC], f32, tag=f"zmm{t}", name=f"zmm{t}")
                    nc.tensor.matmul(
                        ps[:], W1[:, t * 128 : (t + 1) * 128], XT[:], start=True, stop=True
                    )
                    sb = mpool.tile([128, BC], f16, tag=f"z1_{t}", name=f"z1_{t}")
                    nc.scalar.activation(sb[:], ps[:], AF.Relu, bias=b1r[:, t : t + 1])
                    z1t.append(sb)
                zps2 = [
                    mpsum.tile([128, BC], f32, tag=f"zmm{t}", name=f"zmm2{t}")
                    for t in range(8)
                ]
                for k in range(8):
                    for t in range(8):
                        nc.tensor.matmul(
                            zps2[t][:],
                            W2[k][:, t * 128 : (t + 1) * 128],
                            z1t[k][:],
                            start=(k == 0),
                            stop=(k == 7),
                        )
                z2t = []
                for t in range(8):
                    sb = mpool.tile([128, BC], f16, tag=f"z2_{t}", name=f"z2_{t}")
                    nc.scalar.activation(sb[:], zps2[t][:], AF.Relu, bias=b2r[:, t : t + 1])
                    z2t.append(sb)
                zps3 = [
                    mpsum.tile([128, BC], f32, tag=f"zmm{c}", name=f"z3mm{c}")
                    for c in range(NF)
                ]
                for k in range(8):
                    for c in range(NF):
                        nc.tensor.matmul(
                            zps3[c][:],
                            W3[k][:, c * 128 : (c + 1) * 128],
                            z2t[k][:],
                            start=(k == 0),
                            stop=(k == 7),
                        )
                for c in range(NF):
                    sb = cpool.tile([128, BC], f32r, tag=f"zt_{c}", name=f"zt_{c}")
                    nc.scalar.activation(sb[:], zps3[c][:], AF.Relu, bias=b3r[:, c : c + 1])
                    zt.append(sb)

            # ---- PDHG setup ----
            spool = stack.enter_context(tc.tile_pool(name="setup", bufs=1))
            e2_pool = stack.enter_context(tc.tile_pool(name="e2p", bufs=3))
            e2t_pool = stack.enter_context(tc.tile_pool(name="e2tp", bufs=3))
            q_pool = stack.enter_context(tc.tile_pool(name="qp", bufs=3))
            p_pool = stack.enter_context(tc.tile_pool(name="pp", bufs=3))
            sc_pool = stack.enter_context(tc.tile_pool(name="scratch", bufs=3))
            with tc.tile_pool(name="pd_ps", bufs=1, space="PSUM") as ppool:
                # cSZB' = alpha*(S@Z^T - B^T) - atau*(S@1) 1^T   [64, BC] fp16
                ps = ppool.tile([N_COMBOS, BC], f32, tag="py1", name="py1")
                for c in range(NF):
                    nc.tensor.matmul(
                        ps[:], aST[:, c * 64 : (c + 1) * 64], zt[c][:],
                        start=(c == 0), stop=False,
                    )
                naI64 = spool.tile([N_COMBOS, N_COMBOS], f32r, tag="nai64", name="naI64")
                nc.scalar.activation(naI64[:], Ir[:64, :64].bitcast(f32), AF.Copy, scale=-alpha)
                nc.tensor.matmul(ps[:], naI64[:], XT[:], start=False, stop=True)
                cSZB0 = spool.tile([N_COMBOS, BC], f16, tag="cszb0", name="cSZB0")
                nc.scalar.activation(cSZB0[:], ps[:], AF.Copy)
                # cSZB' = cSZB + sbias (per-partition const, folds atau*S@1)
                cSZB = spool.tile([N_COMBOS, BC], f16, tag="cszb", name="cSZB")
                nc.vector.tensor_scalar(
                    cSZB[:], cSZB0[:], sbias[:], 0.0, op0=ALU.add, op1=ALU.add
                )

                # Z per-b in N layout (f32) via PE transposes
                Zf = []
                for b in range(NB):
                    psz = ppool.tile([128, FW], f32r, tag=f"pz{b}", name=f"pz{b}")
                    for c in range(NF):
                        nc.tensor.transpose(
                            psz[:, c * 128 : (c + 1) * 128],
                            zt[c][:, b * 128 : (b + 1) * 128],
                            Ir[:],
                        )
                    zb = spool.tile([128, FW], f32, tag=f"zn{b}", name=f"zn{b}")
                    nc.scalar.activation(zb[:], psz[:].bitcast(f32), AF.Copy)
                    Zf.append(zb)

                # state init
                naZ, E2, E2p, E2T, E2Tp, q, pc = [], [], [], [], [], [], []
                for b in range(NB):
                    t = spool.tile([128, FW], f16, tag=f"naz{b}", name=f"naz{b}")
                    nc.scalar.activation(t[:], Zf[b][:], AF.Copy, scale=-alpha, bias=atau)
                    naZ.append(t)
                    t = e2_pool.tile([128, FW], f16, tag=f"e2{b}", name=f"e2i{b}")
                    nc.vector.tensor_scalar(t[:], Zf[b][:], 2.0 * alpha, -2.0 * atau, op0=ALU.mult, op1=ALU.add)
                    E2.append(t)
                    E2p.append(t)  # E2_{-1} = E2_0
                    # E2T_0 via PE transpose + evac
                    pst = ppool.tile([128, FW], f16, tag=f"pTi{b}", name=f"pTi{b}")
                    for c in range(NF):
                        nc.tensor.transpose(
                            pst[:, c * 128 : (c + 1) * 128],
                            t[:, c * 128 : (c + 1) * 128],
                            I16[:],
                        )
                    tt_ = e2t_pool.tile([128, FW], f16, tag=f"e2t{b}", name=f"e2ti{b}")
                    nc.vector.tensor_copy(tt_[:], pst[:])
                    E2T.append(tt_)
                    E2Tp.append(tt_)
                    tq = q_pool.tile([128, FW], f16, tag=f"q{b}", name=f"qi{b}")
                    nc.gpsimd.memset(tq[:], 0.0)
                    q.append(tq)
                    if not CFG.get("pcnelim", False):
                        tp_ = p_pool.tile([N_COMBOS, 128], f16, tag=f"pc{b}", name=f"pci{b}")
                        nc.vector.tensor_copy(tp_[:], cSZB[:, b * 128 : (b + 1) * 128])
                        pc.append(tp_)

            ps_T = stack.enter_context(tc.tile_pool(name="ps_T", bufs=(2 if CFG.get("sharedT") else CFG.get("bufsT", 1)), space="PSUM"))
            ps_y1 = stack.enter_context(tc.tile_pool(name="ps_y1", bufs=CFG.get("bufsY", 1), space="PSUM"))
            ps_3 = stack.enter_context(tc.tile_pool(name="ps_3", bufs=(2 if CFG.get("shared3") else 1), space="PSUM"))
            ps_j = stack.enter_context(tc.tile_pool(name="ps_j", bufs=1, space="PSUM"))

            # ---- iteration emission (v1 shape: trans+evac right after E2n) ----
            NSTG = 12
            K_OFF = CFG.get("koff", 6)
            temps = [dict(), dict()]
            HALF = FW // 2
            PCN_ELIM = CFG.get("pcnelim", False)
            EH_ROUTE = CFG.get("ehroute", False)

            # Analytic init: h_0 = w_0 + aeb_0 = (atau - aZ) + aZ ... = 0 exactly,
            # so qn_0 = 0; and Eh_0 = -0.5*E2_0 = -aZ + atau = naZ (alias).
            Eh = list(naZ)
            qn_next = []
            for b in range(NB):
                t = q_pool.tile([128, FW], f16, tag=f"q{b}", name=f"qn0{b}")
                nc.gpsimd.memset(t[:], 0.0)
                qn_next.append(t)
            qn = [None, None]
            p_state = []
            for b in range(NB):
                t = p_pool.tile([N_COMBOS, 128], f16, tag=f"p{b}", name=f"pz{b}")
                nc.gpsimd.memset(t[:], 0.0)
                p_state.append(t)

            def emit(it, b, s):
                T = temps[b]
                last = it == N_ITERS - 1
                if s == 0:
                    qn[b] = qn_next[b]
                    # w_{k+1} = qn_k + naZ (Pool) -- for NEXT iter's h
                    if not last:
                        T["w"] = sc_pool.tile([128, FW], f16, tag=f"w{b}", name=f"w{b}")
                        if CFG["w"] == "pool":
                            L(f"w.{b}", nc.gpsimd.tensor_tensor(
                                T["w"][:], qn[b][:], naZ[b][:], ALU.add
                            ))
                        else:
                            L(f"w.{b}", nc.vector.tensor_tensor(T["w"][:], qn[b][:], naZ[b][:], ALU.add))
                elif s == 1:
                    if EH_ROUTE and not last:
                        T["hv"] = sc_pool.tile([128, FW], f16, tag=f"hv{b}", name=f"hv{b}")
                        L(f"hv.{b}", nc.vector.tensor_tensor(T["hv"][:], T["w"][:], Eh[b][:], ALU.add))
                elif s == 2:
                    if CFG.get("earlyps1"):
                        T["ps1"] = ps_y1.tile([N_COMBOS, 128], f32, tag=f"py{b}", name=f"py{b}")
                        L(f"ps1c.{b}", nc.tensor.matmul(
                            T["ps1"][:], I64[:], cSZB[:, b * 128 : (b + 1) * 128],
                            start=True, stop=False,
                        ))
                        L(f"ps1p.{b}", nc.tensor.matmul(
                            T["ps1"][:], I64[:], p_state[b][:], start=False, stop=False,
                        ))
                        for c in range(NF):
                            L(f"ps1b{c}.{b}", nc.tensor.matmul(
                                T["ps1"][:],
                                STb[:, c * 64 : (c + 1) * 64],
                                E2Tp[b][:, c * 128 : (c + 1) * 128],
                                start=False, stop=False,
                            ))
                elif s == 3:
                    pass
                elif s == 4:
                    if CFG.get("earlyps1"):
                        for c in range(NF):
                            L(f"ps1a{c}.{b}", nc.tensor.matmul(
                                T["ps1"][:],
                                STa[:, c * 64 : (c + 1) * 64],
                                E2T[b][:, c * 128 : (c + 1) * 128],
                                start=False, stop=(c == NF - 1),
                            ))
                        return
                    T["ps1"] = ps_y1.tile([N_COMBOS, 128], f32, tag=f"py{b}", name=f"py{b}")
                    if PCN_ELIM:
                        L(f"ps1c.{b}", nc.tensor.matmul(
                            T["ps1"][:], I64[:], cSZB[:, b * 128 : (b + 1) * 128],
                            start=True, stop=False,
                        ))
                        L(f"ps1p.{b}", nc.tensor.matmul(
                            T["ps1"][:], I64[:], p_state[b][:], start=False, stop=False,
                        ))
                    else:
                        L(f"ps1c.{b}", nc.tensor.matmul(
                            T["ps1"][:], I64[:], pc[b][:], start=True, stop=False,
                        ))
                    for c in range(NF):
                        L(f"ps1b{c}.{b}", nc.tensor.matmul(
                            T["ps1"][:],
                            STb[:, c * 64 : (c + 1) * 64],
                            E2Tp[b][:, c * 128 : (c + 1) * 128],
                            start=False, stop=False,
                        ))
                    for c in range(NF):
                        L(f"ps1a{c}.{b}", nc.tensor.matmul(
                            T["ps1"][:],
                            STa[:, c * 64 : (c + 1) * 64],
                            E2T[b][:, c * 128 : (c + 1) * 128],
                            start=False, stop=(c == NF - 1),
                        ))
                elif s == 5:
                    T["p"] = p_pool.tile([N_COMBOS, 128], f16, tag=f"p{b}", name=f"p{b}")
                    hp = tc.high_priority(CFG.get("phprio")) if CFG.get("phprio") else contextlib.nullcontext()
                    with hp:
                        if CFG["p"] == "act":
                            L(f"p.{b}", nc.scalar.activation(T["p"][:], T["ps1"][:], AF.Relu))
                        else:
                            L(f"p.{b}", nc.vector.tensor_scalar_max(T["p"][:], T["ps1"][:], 0.0))
                elif s == 6:
                    if not PCN_ELIM and not last:
                        T["pcn"] = p_pool.tile([N_COMBOS, 128], f16, tag=f"pc{b}", name=f"pc{b}")
                        if CFG["pc"] == "pool":
                            L(f"pcn.{b}", nc.gpsimd.tensor_tensor(
                                T["pcn"][:], T["p"][:], cSZB[:, b * 128 : (b + 1) * 128], ALU.add
                            ))
                        else:
                            L(f"pcn.{b}", nc.vector.tensor_tensor(
                                T["pcn"][:], T["p"][:], cSZB[:, b * 128 : (b + 1) * 128], ALU.add
                            ))
                elif s == 7:
                    T["ns3"] = ps_3.tile([128, FW], f32, tag=("p3" if CFG.get("shared3") else f"p3{b}"), name=f"p3{b}")
                    L(f"mm1.{b}", nc.tensor.matmul(T["ns3"][:], hI16[:], E2[b][:], start=True, stop=False))
                    L(f"mm2.{b}", nc.tensor.matmul(T["ns3"][:], nAI16[:], qn[b][:], start=False, stop=False))
                    hpc = tc.high_priority(CFG.get("chprio")) if CFG.get("chprio") else contextlib.nullcontext()
                    with hpc:
                        L(f"mmC.{b}", nc.tensor.matmul(T["ns3"][:], T["p"][:], AS16[:], start=False, stop=True))
                elif s == 8:
                    T["n2"] = sc_pool.tile([128, 1], f32, tag=f"n2{b}", name=f"n2{b}")
                    if CFG.get("jsbuf"):
                        dsqj = sc_pool.tile([128, FW], f32, tag="dsqj", name=f"dsqj{b}")
                    else:
                        dsqj = ps_j.tile([128, FW], f32, tag="dsqj", name=f"dsqj{b}")
                    L(f"dsq.{b}", nc.scalar.activation(dsqj[:], T["ns3"][:], AF.Square, accum_out=T["n2"][:]))
                elif s == 9:
                    T["nr"] = sc_pool.tile([128, 1], f32, tag=f"nr{b}", name=f"nr{b}")
                    L(f"nr.{b}", nc.scalar.activation(
                        T["nr"][:], T["n2"][:], AF.Abs_reciprocal_sqrt, scale=dsq_scale, bias=1e-6
                    ))
                    T["s2p"] = sc_pool.tile([128, 1], f32, tag=f"s2p{b}", name=f"s2p{b}")
                    L(f"s2p.{b}", nc.scalar.activation(T["s2p"][:], T["nr"][:], AF.Relu, scale=-2.0, bias=2.0))
                elif s == 10:
                    if last:
                        nsa = sc_pool.tile([128, 1], f32, tag=f"nsa{b}", name=f"nsa{b}")
                        nc.vector.tensor_scalar(
                            nsa[:], T["s2p"][:], -0.5 / alpha, 0.0, op0=ALU.mult, op1=ALU.add
                        )
                        xout = sc_pool.tile([128, FW], f32, tag=f"xo{b}", name=f"xo{b}")
                        nc.vector.affine_then_add(
                            xout[:], T["ns3"][:], Zf[b][:], scale=nsa[:], bias=0.0
                        )
                        nc.sync.dma_start(d_out.ap()[b * 128 : (b + 1) * 128, :], xout[:])
                        return
                    T["E2n"] = e2_pool.tile([128, FW], f16, tag=f"e2{b}", name=f"e2n{b}")
                    if CFG["e2n"] == "split":
                        L(f"E2nA.{b}", nc.scalar.activation(
                            T["E2n"][:, 0:HALF], T["ns3"][:, 0:HALF], AF.Copy,
                            scale=T["s2p"][:], bias=-2.0 * atau,
                        ))
                        L(f"E2nB.{b}", nc.vector.tensor_scalar(
                            T["E2n"][:, HALF:FW], T["ns3"][:, HALF:FW], T["s2p"][:],
                            -2.0 * atau, op0=ALU.mult, op1=ALU.add,
                        ))
                    else:
                        L(f"E2n.{b}", nc.scalar.activation(
                            T["E2n"][:], T["ns3"][:], AF.Copy,
                            scale=T["s2p"][:], bias=-2.0 * atau,
                        ))
                    T["psT"] = ps_T.tile([128, FW], f16, tag=("pT" if CFG.get("sharedT") else f"pT{b}"), name=f"pT{b}")
                    hpt = tc.high_priority(CFG.get("thprio")) if CFG.get("thprio") else contextlib.nullcontext()
                    with hpt:
                        for c in range(NF):
                            L(f"tr{c}.{b}", nc.tensor.transpose(
                                T["psT"][:, c * 128 : (c + 1) * 128],
                                T["E2n"][:, c * 128 : (c + 1) * 128],
                                I16[:],
                            ))
                    T["E2Tn"] = e2t_pool.tile([128, FW], f16, tag=f"e2t{b}", name=f"e2tn{b}")
                    hpe = tc.high_priority(CFG.get("ehprio")) if CFG.get("ehprio") else contextlib.nullcontext()
                    with hpe:
                        ev = CFG.get("evac", "dve")
                        if ev == "act":
                            L(f"evac.{b}", nc.scalar.activation(T["E2Tn"][:], T["psT"][:].bitcast(f16), AF.Copy))
                        elif ev == "split":
                            L(f"evacA.{b}", nc.scalar.activation(T["E2Tn"][:, 0:HALF], T["psT"][:, 0:HALF].bitcast(f16), AF.Copy))
                            L(f"evac.{b}", nc.vector.tensor_copy(T["E2Tn"][:, HALF:FW], T["psT"][:, HALF:FW]))
                        else:
                            L(f"evac.{b}", nc.vector.tensor_copy(T["E2Tn"][:], T["psT"][:]))
                    # q-chain for next iter
                    if EH_ROUTE:
                        T["h2"] = sc_pool.tile([128, FW], f16, tag=f"h2{b}", name=f"h2{b}")
                        L(f"h.{b}", nc.vector.tensor_tensor(T["h2"][:], T["hv"][:], T["E2n"][:], ALU.add))
                        qn_next[b] = q_pool.tile([128, FW], f16, tag=f"q{b}", name=f"qn{b}")
                        L(f"qn.{b}", nc.vector.tensor_scalar_max(qn_next[b][:], T["h2"][:], 0.0))
                        T["Ehn"] = sc_pool.tile([128, FW], f16, tag=f"eh{b}", name=f"ehn{b}")
                        L(f"Eh.{b}", nc.vector.tensor_scalar(
                            T["Ehn"][:], T["E2n"][:], -0.5, 0.0, op0=ALU.mult, op1=ALU.add
                        ))
                    else:
                        T["hA"] = sc_pool.tile([128, FW], f16, tag=f"hA{b}", name=f"hA{b}")
                        L(f"hA.{b}", nc.vector.scalar_tensor_tensor(
                            T["hA"][:], E2[b][:], -0.5, T["E2n"][:], op0=ALU.mult, op1=ALU.add
                        ))
                        T["h"] = sc_pool.tile([128, FW], f16, tag=f"h{b}", name=f"h{b}")
                        L(f"h.{b}", nc.vector.tensor_tensor(T["h"][:], T["hA"][:], T["w"][:], ALU.add))
                        qn_next[b] = q_pool.tile([128, FW], f16, tag=f"q{b}", name=f"qn{b}")
                        hpq = tc.high_priority(CFG.get("qhprio")) if CFG.get("qhprio") else contextlib.nullcontext()
                        with hpq:
                            L(f"qn.{b}", nc.vector.tensor_scalar_max(qn_next[b][:], T["h"][:], 0.0))
                elif s == 11:
                    if not last:
                        E2p[b] = E2[b]
                        E2[b] = T["E2n"]
                        E2Tp[b] = E2T[b]
                        E2T[b] = T["E2Tn"]
                        p_state[b] = T["p"]
                        if not PCN_ELIM:
                            pc[b] = T["pcn"]
                        if EH_ROUTE:
                            Eh[b] = T["Ehn"]

            total = N_ITERS * NSTG
            for gs in range(total + K_OFF):
                if gs < total:
                    emit(gs // NSTG, 0, gs % NSTG)
                g1 = gs - K_OFF
                if 0 <= g1 < total:
                    emit(g1 // NSTG, 1, g1 % NSTG)

    nc.finalize()
    return nc


def _get_nc(S: np.ndarray):
    key = (hash(S.tobytes()), tuple(sorted(CFG.items())))
    if key not in _BUILD_CACHE:
        L = _power_L(S)
        tau = 0.9 / L
        sigma = 0.9 / L
        _BUILD_CACHE[key] = (_build_nc(tau, sigma), tau, sigma)
    return _BUILD_CACHE[key]


def _make_in_maps(X, W1, b1, W2, b2, W3, b3, S, tau, sigma):
    f32 = np.float32
    alpha = np.float32(tau) * np.float32(sigma)
    atau = np.float32(alpha) * np.float32(tau)
    a16 = np.float16(alpha).astype(f32)
    Xflat = np.ascontiguousarray(X.reshape(B_FULL, N_COMBOS)).astype(f32)
    S = S.astype(f32)
    aST_full = (alpha * S.T).astype(f32)  # [512, 64]
    aST = np.ascontiguousarray(
        np.concatenate([aST_full[c * 128 : (c + 1) * 128, :] for c in range(NF)], axis=1)
    )
    STa_full = (-S.T).astype(np.float16)
    STa = np.ascontiguousarray(
        np.concatenate([STa_full[c * 128 : (c + 1) * 128, :] for c in range(NF)], axis=1)
    )
    STb_full = (0.5 * S.T).astype(np.float16)
    STb = np.ascontiguousarray(
        np.concatenate([STb_full[c * 128 : (c + 1) * 128, :] for c in range(NF)], axis=1)
    )
    AS16 = np.ascontiguousarray((a16 * S).astype(np.float16))
    I128 = np.eye(128, dtype=f32)
    sbias = np.ascontiguousarray((-atau * S.sum(axis=1)).astype(f32).reshape(N_COMBOS, 1))
    shared = {
        "w1": np.ascontiguousarray(W1.astype(f32)),
        "b1r": np.ascontiguousarray(b1.reshape(8, 128).T).astype(f32),
        "w2": np.ascontiguousarray(W2.astype(np.float16)),
        "b2r": np.ascontiguousarray(b2.reshape(8, 128).T).astype(f32),
        "w3": np.ascontiguousarray(W3.astype(np.float16)),
        "b3r": np.ascontiguousarray(b3.reshape(4, 128).T).astype(f32),
        "ast": aST,
        "sta16": STa,
        "stb16": STb,
        "as16": AS16,
        "nai16": np.ascontiguousarray((-a16 * I128).astype(np.float16)),
        "hi16": np.ascontiguousarray((0.5 * I128).astype(np.float16)),
        "i64_16": np.eye(N_COMBOS, dtype=np.float16),
        "i16": I128.astype(np.float16),
        "identr": I128,
        "sbias": sbias,
    }
    in_maps = []
    for c in range(N_CORES):
        xt = np.ascontiguousarray(Xflat[c * BC : (c + 1) * BC, :].T)
        in_maps.append({**shared, "xt": xt})
    return in_maps


def kernel(X, W1, b1, W2, b2, W3, b3, S, batch_size):
    from concourse.bass_utils import run_bass_kernel_spmd

    X = np.asarray(X)
    S = np.asarray(S)
    nc, tau, sigma = _get_nc(np.ascontiguousarray(S.astype(np.float32)))
    in_maps = _make_in_maps(
        X, np.asarray(W1), np.asarray(b1), np.asarray(W2), np.asarray(b2),
        np.asarray(W3), np.asarray(b3), S, tau, sigma,
    )
    res = run_bass_kernel_spmd(nc, in_maps, core_ids=list(range(N_CORES)))
    out = np.concatenate([res.results[c]["out"] for c in range(N_CORES)], axis=0)
    return out.astype(np.float32)


# revision 6
# speedup vs baseline: 1.0123x; 1.0045x over previous
"""Trainium2 Bass kernel v2 for nn_MatchNet (MLP forward + 60-iter batched PDHG).

Data-parallel over 8 NeuronCores: batch 2048 -> 256 rows/core (2 b-tiles of 128).

v2 reformulation vs baseline: state E2 := -2*alpha*(x - Z + tau) (fp16, N layout)
plus its transpose E2T kept as a state (2 rotating buffers each). The xbar
combination aeb = E2_k - 0.5*E2_{k-1} + atau is never materialized:
  - its transpose enters ps1 via two weight sets (STa = -S^T, STb = 0.5*S^T),
    with the atau*S@1 constant folded into cSZB.
  - the y2-chain uses hA = -0.5*E2_{k-1} + E2_k (one DVE STT op).
This removes the baseline's per-iteration DVE affine (658ns) + ns/s2p ops;
s2p = relu(2 - 2*nr) is computed on Act directly ([128,1] ops are ~free).

Per iteration (tile b):
  w   = q + naZ'         (Pool STT; naZ' = -alpha*Z + atau)
  hA  = -0.5*E2p + E2c   (DVE STT)
  h   = hA + w           (DVE TT)
  qn  = relu(h)          (DVE TSP)
  ps1 = I64@pc + sum_c STb_c@E2Tp_c + sum_c STa_c@E2Tc_c   (PE, all iter-start ready)
  p   = relu(ps1)        (cfg engine)
  pcn = p + cSZB'        (cfg engine)
  NS3 = hI@E2c + nAI@qn + p@AS16          (PE PSUM [128,512] = -alpha*d)
  n2  = sum(NS3^2)       (Act Square, out->PSUM junk, accum_out)
  nr  = rsqrt(n2/t2a2+eps) (Act)
  s2p = relu(2-2*nr)     (Act)
  E2n = s2p*NS3 - 2atau  (Act h1 + DVE h2, halves)
  psT = transpose(E2n)   (PE, per half)
  E2Tn = copy(psT)       (DVE, per half)
  out x = Z - E2/(2 alpha) - tau  (from final NS3: x = Z + (s2p/(-2 alpha))*NS3)
"""

import numpy as np

N_STRUCTS = 512
N_COMBOS = 64
HID = 1024
N_ITERS = 60
N_CORES = 8
B_FULL = 2048
BC = B_FULL // N_CORES  # 256 batch rows per core
NB = BC // 128  # 2 batch sub-tiles
NF = N_STRUCTS // 128  # 4 feature chunks

OPLOG = {}

CFG = {
    "p": "dve",
    "pc": "pool",
    "w": "pool",
    "e2n": "act",
    "halves": False,
    "koff": 6,
    "pcnelim": True,
    "ehroute": False,
    "phprio": 24,
    "warmact": True,
}

_BUILD_CACHE = {}


def _power_L(S: np.ndarray) -> float:
    S = S.astype(np.float32)
    n = S.shape[1]
    v = np.full((n,), 1.0 / np.sqrt(n), np.float32)
    for _ in range(30):
        v2 = (S.T @ (S @ v) + v).astype(np.float32)
        v = (v2 / np.float32(np.linalg.norm(v2))).astype(np.float32)
    L = np.sqrt(np.vdot(v, (S.T @ (S @ v) + v).astype(np.float32)))
    return float(L)


def _build_nc(tau: float, sigma: float):
    import contextlib

    import concourse.bacc as bacc
    import concourse.mybir as mybir
    import concourse.tile as tile

    f32 = mybir.dt.float32
    f32r = mybir.dt.float32r
    f16 = mybir.dt.float16
    AF = mybir.ActivationFunctionType
    ALU = mybir.AluOpType
    alpha = float(np.float32(tau) * np.float32(sigma))
    atau = float(np.float32(alpha) * np.float32(tau))
    t2a2 = float((np.float32(tau) * np.float32(alpha)) ** 2)
    dsq_scale = float(1.0 / t2a2)

    nc = bacc.Bacc("TRN2", target_bir_lowering=False, debug=False)

    def L(label, inst):
        try:
            OPLOG[str(inst.ins.name)] = label
        except Exception:
            pass
        return inst

    def creg(v):
        key = (f32, v)
        if key not in nc.const_aps.aps:
            t = nc.alloc_sbuf_tensor(f"constx-{v}", [128, 1], f32)
            nc.gpsimd.memset(t.ap(), v)
            nc.const_aps.aps[key] = t.ap()
        return v

    creg(atau)
    creg(-2.0 * atau)
    creg(2.0)
    creg(0.0)
    creg(1e-6)

    # ---- DRAM I/O (per-core shapes) ----
    d_XT = nc.dram_tensor("xt", [N_COMBOS, BC], f32r, kind="ExternalInput")
    d_W1 = nc.dram_tensor("w1", [N_COMBOS, HID], f32r, kind="ExternalInput")
    d_b1 = nc.dram_tensor("b1r", [128, 8], f32, kind="ExternalInput")
    d_W2 = nc.dram_tensor("w2", [HID, HID], f16, kind="ExternalInput")
    d_b2 = nc.dram_tensor("b2r", [128, 8], f32, kind="ExternalInput")
    d_W3 = nc.dram_tensor("w3", [HID, N_STRUCTS], f16, kind="ExternalInput")
    d_b3 = nc.dram_tensor("b3r", [128, 4], f32, kind="ExternalInput")
    d_aST = nc.dram_tensor("ast", [128, NF * N_COMBOS], f32r, kind="ExternalInput")
    d_STa = nc.dram_tensor("sta16", [128, NF * N_COMBOS], f16, kind="ExternalInput")
    d_STb = nc.dram_tensor("stb16", [128, NF * N_COMBOS], f16, kind="ExternalInput")
    d_AS16 = nc.dram_tensor("as16", [N_COMBOS, N_STRUCTS], f16, kind="ExternalInput")
    d_nAI16 = nc.dram_tensor("nai16", [128, 128], f16, kind="ExternalInput")
    d_hI16 = nc.dram_tensor("hi16", [128, 128], f16, kind="ExternalInput")
    d_I64 = nc.dram_tensor("i64_16", [N_COMBOS, N_COMBOS], f16, kind="ExternalInput")
    d_I16 = nc.dram_tensor("i16", [128, 128], f16, kind="ExternalInput")
    d_Ir = nc.dram_tensor("identr", [128, 128], f32r, kind="ExternalInput")
    d_sb = nc.dram_tensor("sbias", [N_COMBOS, 1], f32, kind="ExternalInput")
    d_out = nc.dram_tensor("out", [BC, N_STRUCTS], f32, kind="ExternalOutput")

    FW = N_STRUCTS  # 512

    with tile.TileContext(nc) as tc:
        stack = contextlib.ExitStack()
        with stack:
            cpool = stack.enter_context(tc.tile_pool(name="consts", bufs=1))

            def cload(dram, shape, tag, dt):
                t = cpool.tile(shape, dt, tag=tag, name=tag)
                nc.sync.dma_start(t[:], dram.ap())
                return t

            if CFG.get("warmact", False):
                # Dummy activation on a const tile: forces the ACT table load
                # (1283ns) to schedule at t~0 instead of before the first relu.
                wrm = cpool.tile([128, 1], f32, tag="warm", name="warm")
                nc.gpsimd.memset(wrm[:], 0.0)
                nc.scalar.activation(wrm[:], wrm[:], AF.Relu)
                nc.scalar.activation(wrm[:], wrm[:], AF.Square)
                nc.scalar.activation(wrm[:], wrm[:], AF.Abs_reciprocal_sqrt, scale=1.0, bias=1e-6)
            XT = cload(d_XT, [N_COMBOS, BC], "xt", f32r)
            W1 = cload(d_W1, [N_COMBOS, HID], "w1", f32r)
            b1r = cload(d_b1, [128, 8], "b1r", f32)

            # ---- MLP forward (float32r, T layout) ----
            zt = []  # Z^T tiles [128, BC] x4, f32r
            with (
                tc.tile_pool(name="mlp_sb", bufs=1) as mpool,
                tc.tile_pool(name="mlp_ps", bufs=1, space="PSUM") as mpsum,
            ):
                W2 = []
                for k in range(8):
                    t = mpool.tile([128, HID], f16, tag=f"w2_{k}", name=f"w2_{k}")
                    nc.sync.dma_start(t[:], d_W2.ap()[k * 128 : (k + 1) * 128, :])
                    W2.append(t)
                b2r = cload(d_b2, [128, 8], "b2r", f32)
                W3 = []
                for k in range(8):
                    t = mpool.tile([128, N_STRUCTS], f16, tag=f"w3_{k}", name=f"w3_{k}")
                    nc.sync.dma_start(t[:], d_W3.ap()[k * 128 : (k + 1) * 128, :])
                    W3.append(t)
                b3r = cload(d_b3, [128, 4], "b3r", f32)
                aST = cload(d_aST, [128, NF * N_COMBOS], "ast", f32r)
                Ir = cload(d_Ir, [128, 128], "identr", f32r)
                STa = cload(d_STa, [128, NF * N_COMBOS], "sta16", f16)
                STb = cload(d_STb, [128, NF * N_COMBOS], "stb16", f16)
                AS16 = cload(d_AS16, [N_COMBOS, N_STRUCTS], "as16", f16)
                nAI16 = cload(d_nAI16, [128, 128], "nai16", f16)
                hI16 = cload(d_hI16, [128, 128], "hi16", f16)
                I64 = cload(d_I64, [N_COMBOS, N_COMBOS], "i64_16", f16)
                I16 = cload(d_I16, [128, 128], "i16", f16)
                sbias = cload(d_sb, [N_COMBOS, 1], "sbias", f32)
                z1t = []
                for t in range(8):
                    ps = mpsum.tile([128, BC], f32, tag=f"zmm{t}", name=f"zmm{t}")
                    nc.tensor.matmul(
                        ps[:], W1[:, t * 128 : (t + 1) * 128], XT[:], start=True, stop=True
                    )
                    sb = mpool.tile([128, BC], f16, tag=f"z1_{t}", name=f"z1_{t}")
                    nc.scalar.activation(sb[:], ps[:], AF.Relu, bias=b1r[:, t : t + 1])
                    z1t.append(sb)
                zps2 = [
                    mpsum.tile([128, BC], f32, tag=f"zmm{t}", name=f"zmm2{t}")
                    for t in range(8)
                ]
                for k in range(8):
                    for t in range(8):
                        nc.tensor.matmul(
                            zps2[t][:],
                            W2[k][:, t * 128 : (t + 1) * 128],
                            z1t[k][:],
                            start=(k == 0),
                            stop=(k == 7),
                        )
                z2t = []
                for t in range(8):
                    sb = mpool.tile([128, BC], f16, tag=f"z2_{t}", name=f"z2_{t}")
                    nc.scalar.activation(sb[:], zps2[t][:], AF.Relu, bias=b2r[:, t : t + 1])
                    z2t.append(sb)
                zps3 = [
                    mpsum.tile([128, BC], f32, tag=f"zmm{c}", name=f"z3mm{c}")
                    for c in range(NF)
                ]
                for k in range(8):
                    for c in range(NF):
                        nc.tensor.matmul(
                            zps3[c][:],
                            W3[k][:, c * 128 : (c + 1) * 128],
                            z2t[k][:],
                            start=(k == 0),
                            stop=(k == 7),
                        )
                for c in range(NF):
                    sb = cpool.tile([128, BC], f32r, tag=f"zt_{c}", name=f"zt_{c}")
                    nc.scalar.activation(sb[:], zps3[c][:], AF.Relu, bias=b3r[:, c : c + 1])
                    zt.append(sb)

            # ---- PDHG setup ----
            spool = stack.enter_context(tc.tile_pool(name="setup", bufs=1))
            e2_pool = stack.enter_context(tc.tile_pool(name="e2p", bufs=3))
            e2t_pool = stack.enter_context(tc.tile_pool(name="e2tp", bufs=3))
            q_pool = stack.enter_context(tc.tile_pool(name="qp", bufs=3))
            p_pool = stack.enter_context(tc.tile_pool(name="pp", bufs=3))
            sc_pool = stack.enter_context(tc.tile_pool(name="scratch", bufs=3))
            with tc.tile_pool(name="pd_ps", bufs=1, space="PSUM") as ppool:
                # cSZB' = alpha*(S@Z^T - B^T) - atau*(S@1) 1^T   [64, BC] fp16
                ps = ppool.tile([N_COMBOS, BC], f32, tag="py1", name="py1")
                for c in range(NF):
                    nc.tensor.matmul(
                        ps[:], aST[:, c * 64 : (c + 1) * 64], zt[c][:],
                        start=(c == 0), stop=False,
                    )
                naI64 = spool.tile([N_COMBOS, N_COMBOS], f32r, tag="nai64", name="naI64")
                nc.scalar.activation(naI64[:], Ir[:64, :64].bitcast(f32), AF.Copy, scale=-alpha)
                nc.tensor.matmul(ps[:], naI64[:], XT[:], start=False, stop=True)
                cSZB0 = spool.tile([N_COMBOS, BC], f16, tag="cszb0", name="cSZB0")
                nc.scalar.activation(cSZB0[:], ps[:], AF.Copy)
                # cSZB' = cSZB + sbias (per-partition const, folds atau*S@1)
                cSZB = spool.tile([N_COMBOS, BC], f16, tag="cszb", name="cSZB")
                nc.vector.tensor_scalar(
                    cSZB[:], cSZB0[:], sbias[:], 0.0, op0=ALU.add, op1=ALU.add
                )

                # Z per-b in N layout (f32) via PE transposes
                Zf = []
                pszl = []
                for b in range(NB):
                    psz = ppool.tile([128, FW], f32r, tag=f"pz{b}", name=f"pz{b}")
                    pszl.append(psz)
                    for c in range(NF):
                        nc.tensor.transpose(
                            psz[:, c * 128 : (c + 1) * 128],
                            zt[c][:, b * 128 : (b + 1) * 128],
                            Ir[:],
                        )
                    zb = spool.tile([128, FW], f32, tag=f"zn{b}", name=f"zn{b}")
                    if CFG.get("zfdve"):
                        nc.vector.tensor_copy(zb[:], psz[:].bitcast(f32))
                    else:
                        nc.scalar.activation(zb[:], psz[:].bitcast(f32), AF.Copy)
                    Zf.append(zb)

                # state init
                naZ, E2, E2p, E2T, E2Tp, q, pc = [], [], [], [], [], [], []
                for b in range(NB):
                    t = spool.tile([128, FW], f16, tag=f"naz{b}", name=f"naz{b}")
                    zsrc = pszl[b][:].bitcast(f32) if CFG.get("pszinit") else Zf[b][:]
                    nc.scalar.activation(t[:], zsrc, AF.Copy, scale=-alpha, bias=atau)
                    naZ.append(t)
                    t = e2_pool.tile([128, FW], f16, tag=f"e2{b}", name=f"e2i{b}")
                    nc.vector.tensor_scalar(t[:], zsrc, 2.0 * alpha, -2.0 * atau, op0=ALU.mult, op1=ALU.add)
                    E2.append(t)
                    E2p.append(t)  # E2_{-1} = E2_0
                    # E2T_0 via PE transpose + evac
                    pst = ppool.tile([128, FW], f16, tag=f"pTi{b}", name=f"pTi{b}")
                    for c in range(NF):
                        nc.tensor.transpose(
                            pst[:, c * 128 : (c + 1) * 128],
                            t[:, c * 128 : (c + 1) * 128],
                            I16[:],
                        )
                    tt_ = e2t_pool.tile([128, FW], f16, tag=f"e2t{b}", name=f"e2ti{b}")
                    nc.vector.tensor_copy(tt_[:], pst[:])
                    E2T.append(tt_)
                    E2Tp.append(tt_)
                    tq = q_pool.tile([128, FW], f16, tag=f"q{b}", name=f"qi{b}")
                    nc.gpsimd.memset(tq[:], 0.0)
                    q.append(tq)
                    if not CFG.get("pcnelim", False):
                        tp_ = p_pool.tile([N_COMBOS, 128], f16, tag=f"pc{b}", name=f"pci{b}")
                        nc.vector.tensor_copy(tp_[:], cSZB[:, b * 128 : (b + 1) * 128])
                        pc.append(tp_)

            ps_T = stack.enter_context(tc.tile_pool(name="ps_T", bufs=(2 if CFG.get("sharedT") else CFG.get("bufsT", 1)), space="PSUM"))
            ps_y1 = stack.enter_context(tc.tile_pool(name="ps_y1", bufs=CFG.get("bufsY", 1), space="PSUM"))
            ps_3 = stack.enter_context(tc.tile_pool(name="ps_3", bufs=(2 if CFG.get("shared3") else 1), space="PSUM"))
            ps_j = stack.enter_context(tc.tile_pool(name="ps_j", bufs=1, space="PSUM"))

            # ---- iteration emission (v1 shape: trans+evac right after E2n) ----
            NSTG = 12
            K_OFF = CFG.get("koff", 6)
            temps = [dict(), dict()]
            HALF = FW // 2
            PCN_ELIM = CFG.get("pcnelim", False)
            EH_ROUTE = CFG.get("ehroute", False)

            # Analytic init: h_0 = w_0 + aeb_0 = (atau - aZ) + aZ ... = 0 exactly,
            # so qn_0 = 0; and Eh_0 = -0.5*E2_0 = -aZ + atau = naZ (alias).
            Eh = list(naZ)
            qn_next = []
            for b in range(NB):
                t = q_pool.tile([128, FW], f16, tag=f"q{b}", name=f"qn0{b}")
                nc.gpsimd.memset(t[:], 0.0)
                qn_next.append(t)
            qn = [None, None]
            p_state = []
            for b in range(NB):
                t = p_pool.tile([N_COMBOS, 128], f16, tag=f"p{b}", name=f"pz{b}")
                nc.gpsimd.memset(t[:], 0.0)
                p_state.append(t)

            def emit(it, b, s):
                T = temps[b]
                last = it == N_ITERS - 1
                if s == 0:
                    qn[b] = qn_next[b]
                    # w_{k+1} = qn_k + naZ (Pool) -- for NEXT iter's h
                    if not last:
                        T["w"] = sc_pool.tile([128, FW], f16, tag=f"w{b}", name=f"w{b}")
                        if CFG["w"] == "pool":
                            L(f"w.{b}", nc.gpsimd.tensor_tensor(
                                T["w"][:], qn[b][:], naZ[b][:], ALU.add
                            ))
                        else:
                            L(f"w.{b}", nc.vector.tensor_tensor(T["w"][:], qn[b][:], naZ[b][:], ALU.add))
                elif s == 1:
                    if EH_ROUTE and not last:
                        T["hv"] = sc_pool.tile([128, FW], f16, tag=f"hv{b}", name=f"hv{b}")
                        L(f"hv.{b}", nc.vector.tensor_tensor(T["hv"][:], T["w"][:], Eh[b][:], ALU.add))
                elif s == 2:
                    if CFG.get("earlyps1"):
                        T["ps1"] = ps_y1.tile([N_COMBOS, 128], f32, tag=f"py{b}", name=f"py{b}")
                        L(f"ps1c.{b}", nc.tensor.matmul(
                            T["ps1"][:], I64[:], cSZB[:, b * 128 : (b + 1) * 128],
                            start=True, stop=False,
                        ))
                        L(f"ps1p.{b}", nc.tensor.matmul(
                            T["ps1"][:], I64[:], p_state[b][:], start=False, stop=False,
                        ))
                        for c in range(NF):
                            L(f"ps1b{c}.{b}", nc.tensor.matmul(
                                T["ps1"][:],
                                STb[:, c * 64 : (c + 1) * 64],
                                E2Tp[b][:, c * 128 : (c + 1) * 128],
                                start=False, stop=False,
                            ))
                elif s == 3:
                    pass
                elif s == 4:
                    if CFG.get("earlyps1"):
                        for c in range(NF):
                            L(f"ps1a{c}.{b}", nc.tensor.matmul(
                                T["ps1"][:],
                                STa[:, c * 64 : (c + 1) * 64],
                                E2T[b][:, c * 128 : (c + 1) * 128],
                                start=False, stop=(c == NF - 1),
                            ))
                        return
                    T["ps1"] = ps_y1.tile([N_COMBOS, 128], f32, tag=f"py{b}", name=f"py{b}")
                    if PCN_ELIM:
                        L(f"ps1c.{b}", nc.tensor.matmul(
                            T["ps1"][:], I64[:], cSZB[:, b * 128 : (b + 1) * 128],
                            start=True, stop=False,
                        ))
                        L(f"ps1p.{b}", nc.tensor.matmul(
                            T["ps1"][:], I64[:], p_state[b][:], start=False, stop=False,
                        ))
                    else:
                        L(f"ps1c.{b}", nc.tensor.matmul(
                            T["ps1"][:], I64[:], pc[b][:], start=True, stop=False,
                        ))
                    for c in range(NF):
                        L(f"ps1b{c}.{b}", nc.tensor.matmul(
                            T["ps1"][:],
                            STb[:, c * 64 : (c + 1) * 64],
                            E2Tp[b][:, c * 128 : (c + 1) * 128],
                            start=False, stop=False,
                        ))
                    for c in range(NF):
                        L(f"ps1a{c}.{b}", nc.tensor.matmul(
                            T["ps1"][:],
                            STa[:, c * 64 : (c + 1) * 64],
                            E2T[b][:, c * 128 : (c + 1) * 128],
                            start=False, stop=(c == NF - 1),
                        ))
                elif s == 5:
                    T["p"] = p_pool.tile([N_COMBOS, 128], f16, tag=f"p{b}", name=f"p{b}")
                    hp = tc.high_priority(CFG.get("phprio")) if CFG.get("phprio") else contextlib.nullcontext()
                    with hp:
                        if CFG["p"] == "act":
                            L(f"p.{b}", nc.scalar.activation(T["p"][:], T["ps1"][:], AF.Relu))
                        else:
                            L(f"p.{b}", nc.vector.tensor_scalar_max(T["p"][:], T["ps1"][:], 0.0))
                elif s == 6:
                    if not PCN_ELIM and not last:
                        T["pcn"] = p_pool.tile([N_COMBOS, 128], f16, tag=f"pc{b}", name=f"pc{b}")
                        if CFG["pc"] == "pool":
                            L(f"pcn.{b}", nc.gpsimd.tensor_tensor(
                                T["pcn"][:], T["p"][:], cSZB[:, b * 128 : (b + 1) * 128], ALU.add
                            ))
                        else:
                            L(f"pcn.{b}", nc.vector.tensor_tensor(
                                T["pcn"][:], T["p"][:], cSZB[:, b * 128 : (b + 1) * 128], ALU.add
                            ))
                elif s == 7:
                    T["ns3"] = ps_3.tile([128, FW], f32, tag=("p3" if CFG.get("shared3") else f"p3{b}"), name=f"p3{b}")
                    L(f"mm1.{b}", nc.tensor.matmul(T["ns3"][:], hI16[:], E2[b][:], start=True, stop=False))
                    L(f"mm2.{b}", nc.tensor.matmul(T["ns3"][:], nAI16[:], qn[b][:], start=False, stop=False))
                    hpc = tc.high_priority(CFG.get("chprio")) if CFG.get("chprio") else contextlib.nullcontext()
                    with hpc:
                        L(f"mmC.{b}", nc.tensor.matmul(T["ns3"][:], T["p"][:], AS16[:], start=False, stop=True))
                elif s == 8:
                    T["n2"] = sc_pool.tile([128, 1], f32, tag=f"n2{b}", name=f"n2{b}")
                    if CFG.get("jsbuf"):
                        dsqj = sc_pool.tile([128, FW], f32, tag="dsqj", name=f"dsqj{b}")
                    else:
                        dsqj = ps_j.tile([128, FW], f32, tag="dsqj", name=f"dsqj{b}")
                    L(f"dsq.{b}", nc.scalar.activation(dsqj[:], T["ns3"][:], AF.Square, accum_out=T["n2"][:]))
                elif s == 9:
                    T["nr"] = sc_pool.tile([128, 1], f32, tag=f"nr{b}", name=f"nr{b}")
                    L(f"nr.{b}", nc.scalar.activation(
                        T["nr"][:], T["n2"][:], AF.Abs_reciprocal_sqrt, scale=dsq_scale, bias=1e-6
                    ))
                    T["s2p"] = sc_pool.tile([128, 1], f32, tag=f"s2p{b}", name=f"s2p{b}")
                    L(f"s2p.{b}", nc.scalar.activation(T["s2p"][:], T["nr"][:], AF.Relu, scale=-2.0, bias=2.0))
                elif s == 10:
                    if last:
                        nsa = sc_pool.tile([128, 1], f32, tag=f"nsa{b}", name=f"nsa{b}")
                        nc.vector.tensor_scalar(
                            nsa[:], T["s2p"][:], -0.5 / alpha, 0.0, op0=ALU.mult, op1=ALU.add
                        )
                        xout = sc_pool.tile([128, FW], f32, tag=f"xo{b}", name=f"xo{b}")
                        nc.vector.affine_then_add(
                            xout[:], T["ns3"][:], Zf[b][:], scale=nsa[:], bias=0.0
                        )
                        nc.sync.dma_start(d_out.ap()[b * 128 : (b + 1) * 128, :], xout[:])
                        return
                    T["E2n"] = e2_pool.tile([128, FW], f16, tag=f"e2{b}", name=f"e2n{b}")
                    if CFG["e2n"] == "split":
                        L(f"E2nA.{b}", nc.scalar.activation(
                            T["E2n"][:, 0:HALF], T["ns3"][:, 0:HALF], AF.Copy,
                            scale=T["s2p"][:], bias=-2.0 * atau,
                        ))
                        L(f"E2nB.{b}", nc.vector.tensor_scalar(
                            T["E2n"][:, HALF:FW], T["ns3"][:, HALF:FW], T["s2p"][:],
                            -2.0 * atau, op0=ALU.mult, op1=ALU.add,
                        ))
                    else:
                        L(f"E2n.{b}", nc.scalar.activation(
                            T["E2n"][:], T["ns3"][:], AF.Copy,
                            scale=T["s2p"][:], bias=-2.0 * atau,
                        ))
                    T["psT"] = ps_T.tile([128, FW], f16, tag=("pT" if CFG.get("sharedT") else f"pT{b}"), name=f"pT{b}")
                    hpt = tc.high_priority(CFG.get("thprio")) if CFG.get("thprio") else contextlib.nullcontext()
                    with hpt:
                        for c in range(NF):
                            L(f"tr{c}.{b}", nc.tensor.transpose(
                                T["psT"][:, c * 128 : (c + 1) * 128],
                                T["E2n"][:, c * 128 : (c + 1) * 128],
                                I16[:],
                            ))
                    T["E2Tn"] = e2t_pool.tile([128, FW], f16, tag=f"e2t{b}", name=f"e2tn{b}")
                    hpe = tc.high_priority(CFG.get("ehprio")) if CFG.get("ehprio") else contextlib.nullcontext()
                    with hpe:
                        ev = CFG.get("evac", "dve")
                        if ev == "act":
                            L(f"evac.{b}", nc.scalar.activation(T["E2Tn"][:], T["psT"][:].bitcast(f16), AF.Copy))
                        elif ev == "split":
                            L(f"evacA.{b}", nc.scalar.activation(T["E2Tn"][:, 0:HALF], T["psT"][:, 0:HALF].bitcast(f16), AF.Copy))
                            L(f"evac.{b}", nc.vector.tensor_copy(T["E2Tn"][:, HALF:FW], T["psT"][:, HALF:FW]))
                        else:
                            L(f"evac.{b}", nc.vector.tensor_copy(T["E2Tn"][:], T["psT"][:]))
                    # q-chain for next iter
                    if EH_ROUTE:
                        T["h2"] = sc_pool.tile([128, FW], f16, tag=f"h2{b}", name=f"h2{b}")
                        L(f"h.{b}", nc.vector.tensor_tensor(T["h2"][:], T["hv"][:], T["E2n"][:], ALU.add))
                        qn_next[b] = q_pool.tile([128, FW], f16, tag=f"q{b}", name=f"qn{b}")
                        L(f"qn.{b}", nc.vector.tensor_scalar_max(qn_next[b][:], T["h2"][:], 0.0))
                        T["Ehn"] = sc_pool.tile([128, FW], f16, tag=f"eh{b}", name=f"ehn{b}")
                        L(f"Eh.{b}", nc.vector.tensor_scalar(
                            T["Ehn"][:], T["E2n"][:], -0.5, 0.0, op0=ALU.mult, op1=ALU.add
                        ))
                    else:
                        T["hA"] = sc_pool.tile([128, FW], f16, tag=f"hA{b}", name=f"hA{b}")
                        L(f"hA.{b}", nc.vector.scalar_tensor_tensor(
                            T["hA"][:], E2[b][:], -0.5, T["E2n"][:], op0=ALU.mult, op1=ALU.add
                        ))
                        T["h"] = sc_pool.tile([128, FW], f16, tag=f"h{b}", name=f"h{b}")
                        L(f"h.{b}", nc.vector.tensor_tensor(T["h"][:], T["hA"][:], T["w"][:], ALU.add))
                        qn_next[b] = q_pool.tile([128, FW], f16, tag=f"q{b}", name=f"qn{b}")
                        hpq = tc.high_priority(CFG.get("qhprio")) if CFG.get("qhprio") else contextlib.nullcontext()
                        with hpq:
                            L(f"qn.{b}", nc.vector.tensor_scalar_max(qn_next[b][:], T["h"][:], 0.0))
                elif s == 11:
                    if not last:
                        E2p[b] = E2[b]
                        E2[b] = T["E2n"]
                        E2Tp[b] = E2T[b]
                        E2T[b] = T["E2Tn"]
                        p_state[b] = T["p"]
                        if not PCN_ELIM:
                            pc[b] = T["pcn"]
                        if EH_ROUTE:
                            Eh[b] = T["Ehn"]

            total = N_ITERS * NSTG
            for gs in range(total + K_OFF):
                if gs < total:
                    emit(gs // NSTG, 0, gs % NSTG)
                g1 = gs - K_OFF
                if 0 <= g1 < total:
                    emit(g1 // NSTG, 1, g1 % NSTG)

    nc.finalize()
    return nc


def _get_nc(S: np.ndarray):
    key = (hash(S.tobytes()), tuple(sorted(CFG.items())))
    if key not in _BUILD_CACHE:
        L = _power_L(S)
        tau = 0.9 / L
        sigma = 0.9 / L
        _BUILD_CACHE[key] = (_build_nc(tau, sigma), tau, sigma)
    return _BUILD_CACHE[key]


def _make_in_maps(X, W1, b1, W2, b2, W3, b3, S, tau, sigma):
    f32 = np.float32
    alpha = np.float32(tau) * np.float32(sigma)
    atau = np.float32(alpha) * np.float32(tau)
    a16 = np.float16(alpha).astype(f32)
    Xflat = np.ascontiguousarray(X.reshape(B_FULL, N_COMBOS)).astype(f32)
    S = S.astype(f32)
    aST_full = (alpha * S.T).astype(f32)  # [512, 64]
    aST = np.ascontiguousarray(
        np.concatenate([aST_full[c * 128 : (c + 1) * 128, :] for c in range(NF)], axis=1)
    )
    STa_full = (-S.T).astype(np.float16)
    STa = np.ascontiguousarray(
        np.concatenate([STa_full[c * 128 : (c + 1) * 128, :] for c in range(NF)], axis=1)
    )
    STb_full = (0.5 * S.T).astype(np.float16)
    STb = np.ascontiguousarray(
        np.concatenate([STb_full[c * 128 : (c + 1) * 128, :] for c in range(NF)], axis=1)
    )
    AS16 = np.ascontiguousarray((a16 * S).astype(np.float16))
    I128 = np.eye(128, dtype=f32)
    sbias = np.ascontiguousarray((-atau * S.sum(axis=1)).astype(f32).reshape(N_COMBOS, 1))
    shared = {
        "w1": np.ascontiguousarray(W1.astype(f32)),
        "b1r": np.ascontiguousarray(b1.reshape(8, 128).T).astype(f32),
        "w2": np.ascontiguousarray(W2.astype(np.float16)),
        "b2r": np.ascontiguousarray(b2.reshape(8, 128).T).astype(f32),
        "w3": np.ascontiguousarray(W3.astype(np.float16)),
        "b3r": np.ascontiguousarray(b3.reshape(4, 128).T).astype(f32),
        "ast": aST,
        "sta16": STa,
        "stb16": STb,
        "as16": AS16,
        "nai16": np.ascontiguousarray((-a16 * I128).astype(np.float16)),
        "hi16": np.ascontiguousarray((0.5 * I128).astype(np.float16)),
        "i64_16": np.eye(N_COMBOS, dtype=np.float16),
        "i16": I128.astype(np.float16),
        "identr": I128,
        "sbias": sbias,
    }
    in_maps = []
    for c in range(N_CORES):
        xt = np.ascontiguousarray(Xflat[c * BC : (c + 1) * BC, :].T)
        in_maps.append({**shared, "xt": xt})
    return in_maps


def kernel(X, W1, b1, W2, b2, W3, b3, S, batch_size):
    from concourse.bass_utils import run_bass_kernel_spmd

    X = np.asarray(X)
    S = np.asarray(S)
    nc, tau, sigma = _get_nc(np.ascontiguousarray(S.astype(np.float32)))
    in_maps = _make_in_maps(
        X, np.asarray(W1), np.asarray(b1), np.asarray(W2), np.asarray(b2),
        np.asarray(W3), np.asarray(b3), S, tau, sigma,
    )
    res = run_bass_kernel_spmd(nc, in_maps, core_ids=list(range(N_CORES)))
    out = np.concatenate([res.results[c]["out"] for c in range(N_CORES)], axis=0)
    return out.astype(np.float32)


# revision 7
# speedup vs baseline: 1.0152x; 1.0029x over previous
"""Trainium2 Bass kernel v2 for nn_MatchNet (MLP forward + 60-iter batched PDHG).

Data-parallel over 8 NeuronCores: batch 2048 -> 256 rows/core (2 b-tiles of 128).

v2 reformulation vs baseline: state E2 := -2*alpha*(x - Z + tau) (fp16, N layout)
plus its transpose E2T kept as a state (2 rotating buffers each). The xbar
combination aeb = E2_k - 0.5*E2_{k-1} + atau is never materialized:
  - its transpose enters ps1 via two weight sets (STa = -S^T, STb = 0.5*S^T),
    with the atau*S@1 constant folded into cSZB.
  - the y2-chain uses hA = -0.5*E2_{k-1} + E2_k (one DVE STT op).
This removes the baseline's per-iteration DVE affine (658ns) + ns/s2p ops;
s2p = relu(2 - 2*nr) is computed on Act directly ([128,1] ops are ~free).

Per iteration (tile b):
  w   = q + naZ'         (Pool STT; naZ' = -alpha*Z + atau)
  hA  = -0.5*E2p + E2c   (DVE STT)
  h   = hA + w           (DVE TT)
  qn  = relu(h)          (DVE TSP)
  ps1 = I64@pc + sum_c STb_c@E2Tp_c + sum_c STa_c@E2Tc_c   (PE, all iter-start ready)
  p   = relu(ps1)        (cfg engine)
  pcn = p + cSZB'        (cfg engine)
  NS3 = hI@E2c + nAI@qn + p@AS16          (PE PSUM [128,512] = -alpha*d)
  n2  = sum(NS3^2)       (Act Square, out->PSUM junk, accum_out)
  nr  = rsqrt(n2/t2a2+eps) (Act)
  s2p = relu(2-2*nr)     (Act)
  E2n = s2p*NS3 - 2atau  (Act h1 + DVE h2, halves)
  psT = transpose(E2n)   (PE, per half)
  E2Tn = copy(psT)       (DVE, per half)
  out x = Z - E2/(2 alpha) - tau  (from final NS3: x = Z + (s2p/(-2 alpha))*NS3)
"""

import numpy as np

N_STRUCTS = 512
N_COMBOS = 64
HID = 1024
N_ITERS = 60
N_CORES = 8
B_FULL = 2048
BC = B_FULL // N_CORES  # 256 batch rows per core
NB = BC // 128  # 2 batch sub-tiles
NF = N_STRUCTS // 128  # 4 feature chunks

OPLOG = {}

CFG = {
    "p": "dve",
    "pc": "pool",
    "w": "pool",
    "e2n": "act",
    "halves": False,
    "koff": 6,
    "pcnelim": True,
    "ehroute": False,
    "phprio": 24,
    "warmact": True,
    "w1split": True,
}

_BUILD_CACHE = {}


def _power_L(S: np.ndarray) -> float:
    S = S.astype(np.float32)
    n = S.shape[1]
    v = np.full((n,), 1.0 / np.sqrt(n), np.float32)
    for _ in range(30):
        v2 = (S.T @ (S @ v) + v).astype(np.float32)
        v = (v2 / np.float32(np.linalg.norm(v2))).astype(np.float32)
    L = np.sqrt(np.vdot(v, (S.T @ (S @ v) + v).astype(np.float32)))
    return float(L)


def _build_nc(tau: float, sigma: float):
    import contextlib

    import concourse.bacc as bacc
    import concourse.mybir as mybir
    import concourse.tile as tile

    f32 = mybir.dt.float32
    f32r = mybir.dt.float32r
    f16 = mybir.dt.float16
    AF = mybir.ActivationFunctionType
    ALU = mybir.AluOpType
    alpha = float(np.float32(tau) * np.float32(sigma))
    atau = float(np.float32(alpha) * np.float32(tau))
    t2a2 = float((np.float32(tau) * np.float32(alpha)) ** 2)
    dsq_scale = float(1.0 / t2a2)

    nc = bacc.Bacc("TRN2", target_bir_lowering=False, debug=False)

    def L(label, inst):
        try:
            OPLOG[str(inst.ins.name)] = label
        except Exception:
            pass
        return inst

    def creg(v):
        key = (f32, v)
        if key not in nc.const_aps.aps:
            t = nc.alloc_sbuf_tensor(f"constx-{v}", [128, 1], f32)
            nc.gpsimd.memset(t.ap(), v)
            nc.const_aps.aps[key] = t.ap()
        return v

    creg(atau)
    creg(-2.0 * atau)
    creg(2.0)
    creg(0.0)
    creg(1e-6)

    # ---- DRAM I/O (per-core shapes) ----
    d_XT = nc.dram_tensor("xt", [N_COMBOS, BC], f32r, kind="ExternalInput")
    d_W1 = nc.dram_tensor("w1", [N_COMBOS, HID], f32r, kind="ExternalInput")
    d_b1 = nc.dram_tensor("b1r", [128, 8], f32, kind="ExternalInput")
    d_W2 = nc.dram_tensor("w2", [HID, HID], f16, kind="ExternalInput")
    d_b2 = nc.dram_tensor("b2r", [128, 8], f32, kind="ExternalInput")
    d_W3 = nc.dram_tensor("w3", [HID, N_STRUCTS], f16, kind="ExternalInput")
    d_b3 = nc.dram_tensor("b3r", [128, 4], f32, kind="ExternalInput")
    d_aST = nc.dram_tensor("ast", [128, NF * N_COMBOS], f32r, kind="ExternalInput")
    d_STa = nc.dram_tensor("sta16", [128, NF * N_COMBOS], f16, kind="ExternalInput")
    d_STb = nc.dram_tensor("stb16", [128, NF * N_COMBOS], f16, kind="ExternalInput")
    d_AS16 = nc.dram_tensor("as16", [N_COMBOS, N_STRUCTS], f16, kind="ExternalInput")
    d_nAI16 = nc.dram_tensor("nai16", [128, 128], f16, kind="ExternalInput")
    d_hI16 = nc.dram_tensor("hi16", [128, 128], f16, kind="ExternalInput")
    d_I64 = nc.dram_tensor("i64_16", [N_COMBOS, N_COMBOS], f16, kind="ExternalInput")
    d_I16 = nc.dram_tensor("i16", [128, 128], f16, kind="ExternalInput")
    d_Ir = nc.dram_tensor("identr", [128, 128], f32r, kind="ExternalInput")
    d_sb = nc.dram_tensor("sbias", [N_COMBOS, 1], f32, kind="ExternalInput")
    d_out = nc.dram_tensor("out", [BC, N_STRUCTS], f32, kind="ExternalOutput")

    FW = N_STRUCTS  # 512

    with tile.TileContext(nc) as tc:
        stack = contextlib.ExitStack()
        with stack:
            cpool = stack.enter_context(tc.tile_pool(name="consts", bufs=1))

            def cload(dram, shape, tag, dt):
                t = cpool.tile(shape, dt, tag=tag, name=tag)
                nc.sync.dma_start(t[:], dram.ap())
                return t

            if CFG.get("warmact", False):
                # Dummy activation on a const tile: forces the ACT table load
                # (1283ns) to schedule at t~0 instead of before the first relu.
                wrm = cpool.tile([128, 1], f32, tag="warm", name="warm")
                nc.gpsimd.memset(wrm[:], 0.0)
                nc.scalar.activation(wrm[:], wrm[:], AF.Relu)
                nc.scalar.activation(wrm[:], wrm[:], AF.Square)
                nc.scalar.activation(wrm[:], wrm[:], AF.Abs_reciprocal_sqrt, scale=1.0, bias=1e-6)
            XT = cload(d_XT, [N_COMBOS, BC], "xt", f32r)
            if CFG.get("w1split", False):
                W1 = cpool.tile([N_COMBOS, HID], f32r, tag="w1", name="w1")
                nc.sync.dma_start(W1[:, 0:512], d_W1.ap()[:, 0:512])
                nc.sync.dma_start(W1[:, 512:HID], d_W1.ap()[:, 512:HID])
            else:
                W1 = cload(d_W1, [N_COMBOS, HID], "w1", f32r)
            b1r = cload(d_b1, [128, 8], "b1r", f32)

            # ---- MLP forward (float32r, T layout) ----
            zt = []  # Z^T tiles [128, BC] x4, f32r
            with (
                tc.tile_pool(name="mlp_sb", bufs=1) as mpool,
                tc.tile_pool(name="mlp_ps", bufs=1, space="PSUM") as mpsum,
            ):
                W2 = []
                for k in range(8):
                    t = mpool.tile([128, HID], f16, tag=f"w2_{k}", name=f"w2_{k}")
                    nc.sync.dma_start(t[:], d_W2.ap()[k * 128 : (k + 1) * 128, :])
                    W2.append(t)
                b2r = cload(d_b2, [128, 8], "b2r", f32)
                W3 = []
                for k in range(8):
                    t = mpool.tile([128, N_STRUCTS], f16, tag=f"w3_{k}", name=f"w3_{k}")
                    nc.sync.dma_start(t[:], d_W3.ap()[k * 128 : (k + 1) * 128, :])
                    W3.append(t)
                b3r = cload(d_b3, [128, 4], "b3r", f32)
                aST = cload(d_aST, [128, NF * N_COMBOS], "ast", f32r)
                Ir = cload(d_Ir, [128, 128], "identr", f32r)
                STa = cload(d_STa, [128, NF * N_COMBOS], "sta16", f16)
                STb = cload(d_STb, [128, NF * N_COMBOS], "stb16", f16)
                AS16 = cload(d_AS16, [N_COMBOS, N_STRUCTS], "as16", f16)
                nAI16 = cload(d_nAI16, [128, 128], "nai16", f16)
                hI16 = cload(d_hI16, [128, 128], "hi16", f16)
                I64 = cload(d_I64, [N_COMBOS, N_COMBOS], "i64_16", f16)
                I16 = cload(d_I16, [128, 128], "i16", f16)
                sbias = cload(d_sb, [N_COMBOS, 1], "sbias", f32)
                z1t = []
                for t in range(8):
                    ps = mpsum.tile([128, BC], f32, tag=f"zmm{t}", name=f"zmm{t}")
                    nc.tensor.matmul(
                        ps[:], W1[:, t * 128 : (t + 1) * 128], XT[:], start=True, stop=True
                    )
                    sb = mpool.tile([128, BC], f16, tag=f"z1_{t}", name=f"z1_{t}")
                    nc.scalar.activation(sb[:], ps[:], AF.Relu, bias=b1r[:, t : t + 1])
                    z1t.append(sb)
                zps2 = [
                    mpsum.tile([128, BC], f32, tag=f"zmm{t}", name=f"zmm2{t}")
                    for t in range(8)
                ]
                for k in range(8):
                    for t in range(8):
                        nc.tensor.matmul(
                            zps2[t][:],
                            W2[k][:, t * 128 : (t + 1) * 128],
                            z1t[k][:],
                            start=(k == 0),
                            stop=(k == 7),
                        )
                z2t = []
                for t in range(8):
                    sb = mpool.tile([128, BC], f16, tag=f"z2_{t}", name=f"z2_{t}")
                    nc.scalar.activation(sb[:], zps2[t][:], AF.Relu, bias=b2r[:, t : t + 1])
                    z2t.append(sb)
                zps3 = [
                    mpsum.tile([128, BC], f32, tag=f"zmm{c}", name=f"z3mm{c}")
                    for c in range(NF)
                ]
                for k in range(8):
                    for c in range(NF):
                        nc.tensor.matmul(
                            zps3[c][:],
                            W3[k][:, c * 128 : (c + 1) * 128],
                            z2t[k][:],
                            start=(k == 0),
                            stop=(k == 7),
                        )
                for c in range(NF):
                    sb = cpool.tile([128, BC], f32r, tag=f"zt_{c}", name=f"zt_{c}")
                    nc.scalar.activation(sb[:], zps3[c][:], AF.Relu, bias=b3r[:, c : c + 1])
                    zt.append(sb)

            # ---- PDHG setup ----
            spool = stack.enter_context(tc.tile_pool(name="setup", bufs=1))
            e2_pool = stack.enter_context(tc.tile_pool(name="e2p", bufs=3))
            e2t_pool = stack.enter_context(tc.tile_pool(name="e2tp", bufs=3))
            q_pool = stack.enter_context(tc.tile_pool(name="qp", bufs=3))
            p_pool = stack.enter_context(tc.tile_pool(name="pp", bufs=3))
            sc_pool = stack.enter_context(tc.tile_pool(name="scratch", bufs=3))
            with tc.tile_pool(name="pd_ps", bufs=1, space="PSUM") as ppool:
                # cSZB' = alpha*(S@Z^T - B^T) - atau*(S@1) 1^T   [64, BC] fp16
                ps = ppool.tile([N_COMBOS, BC], f32, tag="py1", name="py1")
                for c in range(NF):
                    nc.tensor.matmul(
                        ps[:], aST[:, c * 64 : (c + 1) * 64], zt[c][:],
                        start=(c == 0), stop=False,
                    )
                naI64 = spool.tile([N_COMBOS, N_COMBOS], f32r, tag="nai64", name="naI64")
                nc.scalar.activation(naI64[:], Ir[:64, :64].bitcast(f32), AF.Copy, scale=-alpha)
                nc.tensor.matmul(ps[:], naI64[:], XT[:], start=False, stop=True)
                cSZB0 = spool.tile([N_COMBOS, BC], f16, tag="cszb0", name="cSZB0")
                nc.scalar.activation(cSZB0[:], ps[:], AF.Copy)
                # cSZB' = cSZB + sbias (per-partition const, folds atau*S@1)
                cSZB = spool.tile([N_COMBOS, BC], f16, tag="cszb", name="cSZB")
                nc.vector.tensor_scalar(
                    cSZB[:], cSZB0[:], sbias[:], 0.0, op0=ALU.add, op1=ALU.add
                )

                # Z per-b in N layout (f32) via PE transposes
                Zf = []
                pszl = []
                for b in range(NB):
                    psz = ppool.tile([128, FW], f32r, tag=f"pz{b}", name=f"pz{b}")
                    pszl.append(psz)
                    for c in range(NF):
                        nc.tensor.transpose(
                            psz[:, c * 128 : (c + 1) * 128],
                            zt[c][:, b * 128 : (b + 1) * 128],
                            Ir[:],
                        )
                    zb = spool.tile([128, FW], f32, tag=f"zn{b}", name=f"zn{b}")
                    if CFG.get("zfdve"):
                        nc.vector.tensor_copy(zb[:], psz[:].bitcast(f32))
                    else:
                        nc.scalar.activation(zb[:], psz[:].bitcast(f32), AF.Copy)
                    Zf.append(zb)

                # state init
                naZ, E2, E2p, E2T, E2Tp, q, pc = [], [], [], [], [], [], []
                for b in range(NB):
                    t = spool.tile([128, FW], f16, tag=f"naz{b}", name=f"naz{b}")
                    zsrc = pszl[b][:].bitcast(f32) if CFG.get("pszinit") else Zf[b][:]
                    nc.scalar.activation(t[:], zsrc, AF.Copy, scale=-alpha, bias=atau)
                    naZ.append(t)
                    t = e2_pool.tile([128, FW], f16, tag=f"e2{b}", name=f"e2i{b}")
                    nc.vector.tensor_scalar(t[:], zsrc, 2.0 * alpha, -2.0 * atau, op0=ALU.mult, op1=ALU.add)
                    E2.append(t)
                    E2p.append(t)  # E2_{-1} = E2_0
                    # E2T_0 via PE transpose + evac
                    pst = ppool.tile([128, FW], f16, tag=f"pTi{b}", name=f"pTi{b}")
                    for c in range(NF):
                        nc.tensor.transpose(
                            pst[:, c * 128 : (c + 1) * 128],
                            t[:, c * 128 : (c + 1) * 128],
                            I16[:],
                        )
                    tt_ = e2t_pool.tile([128, FW], f16, tag=f"e2t{b}", name=f"e2ti{b}")
                    nc.vector.tensor_copy(tt_[:], pst[:])
                    E2T.append(tt_)
                    E2Tp.append(tt_)
                    tq = q_pool.tile([128, FW], f16, tag=f"q{b}", name=f"qi{b}")
                    nc.gpsimd.memset(tq[:], 0.0)
                    q.append(tq)
                    if not CFG.get("pcnelim", False):
                        tp_ = p_pool.tile([N_COMBOS, 128], f16, tag=f"pc{b}", name=f"pci{b}")
                        nc.vector.tensor_copy(tp_[:], cSZB[:, b * 128 : (b + 1) * 128])
                        pc.append(tp_)

            ps_T = stack.enter_context(tc.tile_pool(name="ps_T", bufs=(2 if CFG.get("sharedT") else CFG.get("bufsT", 1)), space="PSUM"))
            ps_y1 = stack.enter_context(tc.tile_pool(name="ps_y1", bufs=CFG.get("bufsY", 1), space="PSUM"))
            ps_3 = stack.enter_context(tc.tile_pool(name="ps_3", bufs=(2 if CFG.get("shared3") else 1), space="PSUM"))
            ps_j = stack.enter_context(tc.tile_pool(name="ps_j", bufs=1, space="PSUM"))

            # ---- iteration emission (v1 shape: trans+evac right after E2n) ----
            NSTG = 12
            K_OFF = CFG.get("koff", 6)
            temps = [dict(), dict()]
            HALF = FW // 2
            PCN_ELIM = CFG.get("pcnelim", False)
            EH_ROUTE = CFG.get("ehroute", False)

            # Analytic init: h_0 = w_0 + aeb_0 = (atau - aZ) + aZ ... = 0 exactly,
            # so qn_0 = 0; and Eh_0 = -0.5*E2_0 = -aZ + atau = naZ (alias).
            Eh = list(naZ)
            qn_next = []
            for b in range(NB):
                t = q_pool.tile([128, FW], f16, tag=f"q{b}", name=f"qn0{b}")
                nc.gpsimd.memset(t[:], 0.0)
                qn_next.append(t)
            qn = [None, None]
            p_state = []
            for b in range(NB):
                t = p_pool.tile([N_COMBOS, 128], f16, tag=f"p{b}", name=f"pz{b}")
                nc.gpsimd.memset(t[:], 0.0)
                p_state.append(t)

            def emit(it, b, s):
                T = temps[b]
                last = it == N_ITERS - 1
                if s == 0:
                    qn[b] = qn_next[b]
                    # w_{k+1} = qn_k + naZ (Pool) -- for NEXT iter's h
                    if not last:
                        T["w"] = sc_pool.tile([128, FW], f16, tag=f"w{b}", name=f"w{b}")
                        if CFG["w"] == "pool":
                            L(f"w.{b}", nc.gpsimd.tensor_tensor(
                                T["w"][:], qn[b][:], naZ[b][:], ALU.add
                            ))
                        else:
                            L(f"w.{b}", nc.vector.tensor_tensor(T["w"][:], qn[b][:], naZ[b][:], ALU.add))
                elif s == 1:
                    if EH_ROUTE and not last:
                        T["hv"] = sc_pool.tile([128, FW], f16, tag=f"hv{b}", name=f"hv{b}")
                        L(f"hv.{b}", nc.vector.tensor_tensor(T["hv"][:], T["w"][:], Eh[b][:], ALU.add))
                elif s == 2:
                    if CFG.get("earlyps1"):
                        T["ps1"] = ps_y1.tile([N_COMBOS, 128], f32, tag=f"py{b}", name=f"py{b}")
                        L(f"ps1c.{b}", nc.tensor.matmul(
                            T["ps1"][:], I64[:], cSZB[:, b * 128 : (b + 1) * 128],
                            start=True, stop=False,
                        ))
                        L(f"ps1p.{b}", nc.tensor.matmul(
                            T["ps1"][:], I64[:], p_state[b][:], start=False, stop=False,
                        ))
                        for c in range(NF):
                            L(f"ps1b{c}.{b}", nc.tensor.matmul(
                                T["ps1"][:],
                                STb[:, c * 64 : (c + 1) * 64],
                                E2Tp[b][:, c * 128 : (c + 1) * 128],
                                start=False, stop=False,
                            ))
                elif s == 3:
                    pass
                elif s == 4:
                    if CFG.get("earlyps1"):
                        for c in range(NF):
                            L(f"ps1a{c}.{b}", nc.tensor.matmul(
                                T["ps1"][:],
                                STa[:, c * 64 : (c + 1) * 64],
                                E2T[b][:, c * 128 : (c + 1) * 128],
                                start=False, stop=(c == NF - 1),
                            ))
                        return
                    T["ps1"] = ps_y1.tile([N_COMBOS, 128], f32, tag=f"py{b}", name=f"py{b}")
                    if PCN_ELIM:
                        L(f"ps1c.{b}", nc.tensor.matmul(
                            T["ps1"][:], I64[:], cSZB[:, b * 128 : (b + 1) * 128],
                            start=True, stop=False,
                        ))
                        L(f"ps1p.{b}", nc.tensor.matmul(
                            T["ps1"][:], I64[:], p_state[b][:], start=False, stop=False,
                        ))
                    else:
                        L(f"ps1c.{b}", nc.tensor.matmul(
                            T["ps1"][:], I64[:], pc[b][:], start=True, stop=False,
                        ))
                    for c in range(NF):
                        L(f"ps1b{c}.{b}", nc.tensor.matmul(
                            T["ps1"][:],
                            STb[:, c * 64 : (c + 1) * 64],
                            E2Tp[b][:, c * 128 : (c + 1) * 128],
                            start=False, stop=False,
                        ))
                    for c in range(NF):
                        L(f"ps1a{c}.{b}", nc.tensor.matmul(
                            T["ps1"][:],
                            STa[:, c * 64 : (c + 1) * 64],
                            E2T[b][:, c * 128 : (c + 1) * 128],
                            start=False, stop=(c == NF - 1),
                        ))
                elif s == 5:
                    T["p"] = p_pool.tile([N_COMBOS, 128], f16, tag=f"p{b}", name=f"p{b}")
                    hp = tc.high_priority(CFG.get("phprio")) if CFG.get("phprio") else contextlib.nullcontext()
                    with hp:
                        if CFG["p"] == "act":
                            L(f"p.{b}", nc.scalar.activation(T["p"][:], T["ps1"][:], AF.Relu))
                        else:
                            L(f"p.{b}", nc.vector.tensor_scalar_max(T["p"][:], T["ps1"][:], 0.0))
                elif s == 6:
                    if not PCN_ELIM and not last:
                        T["pcn"] = p_pool.tile([N_COMBOS, 128], f16, tag=f"pc{b}", name=f"pc{b}")
                        if CFG["pc"] == "pool":
                            L(f"pcn.{b}", nc.gpsimd.tensor_tensor(
                                T["pcn"][:], T["p"][:], cSZB[:, b * 128 : (b + 1) * 128], ALU.add
                            ))
                        else:
                            L(f"pcn.{b}", nc.vector.tensor_tensor(
                                T["pcn"][:], T["p"][:], cSZB[:, b * 128 : (b + 1) * 128], ALU.add
                            ))
                elif s == 7:
                    T["ns3"] = ps_3.tile([128, FW], f32, tag=("p3" if CFG.get("shared3") else f"p3{b}"), name=f"p3{b}")
                    L(f"mm1.{b}", nc.tensor.matmul(T["ns3"][:], hI16[:], E2[b][:], start=True, stop=False))
                    L(f"mm2.{b}", nc.tensor.matmul(T["ns3"][:], nAI16[:], qn[b][:], start=False, stop=False))
                    hpc = tc.high_priority(CFG.get("chprio")) if CFG.get("chprio") else contextlib.nullcontext()
                    with hpc:
                        L(f"mmC.{b}", nc.tensor.matmul(T["ns3"][:], T["p"][:], AS16[:], start=False, stop=True))
                elif s == 8:
                    T["n2"] = sc_pool.tile([128, 1], f32, tag=f"n2{b}", name=f"n2{b}")
                    if CFG.get("jsbuf"):
                        dsqj = sc_pool.tile([128, FW], f32, tag="dsqj", name=f"dsqj{b}")
                    else:
                        dsqj = ps_j.tile([128, FW], f32, tag="dsqj", name=f"dsqj{b}")
                    L(f"dsq.{b}", nc.scalar.activation(dsqj[:], T["ns3"][:], AF.Square, accum_out=T["n2"][:]))
                elif s == 9:
                    T["nr"] = sc_pool.tile([128, 1], f32, tag=f"nr{b}", name=f"nr{b}")
                    L(f"nr.{b}", nc.scalar.activation(
                        T["nr"][:], T["n2"][:], AF.Abs_reciprocal_sqrt, scale=dsq_scale, bias=1e-6
                    ))
                    T["s2p"] = sc_pool.tile([128, 1], f32, tag=f"s2p{b}", name=f"s2p{b}")
                    L(f"s2p.{b}", nc.scalar.activation(T["s2p"][:], T["nr"][:], AF.Relu, scale=-2.0, bias=2.0))
                elif s == 10:
                    if last:
                        nsa = sc_pool.tile([128, 1], f32, tag=f"nsa{b}", name=f"nsa{b}")
                        nc.vector.tensor_scalar(
                            nsa[:], T["s2p"][:], -0.5 / alpha, 0.0, op0=ALU.mult, op1=ALU.add
                        )
                        xout = sc_pool.tile([128, FW], f32, tag=f"xo{b}", name=f"xo{b}")
                        nc.vector.affine_then_add(
                            xout[:], T["ns3"][:], Zf[b][:], scale=nsa[:], bias=0.0
                        )
                        nc.sync.dma_start(d_out.ap()[b * 128 : (b + 1) * 128, :], xout[:])
                        return
                    T["E2n"] = e2_pool.tile([128, FW], f16, tag=f"e2{b}", name=f"e2n{b}")
                    if CFG["e2n"] == "split":
                        L(f"E2nA.{b}", nc.scalar.activation(
                            T["E2n"][:, 0:HALF], T["ns3"][:, 0:HALF], AF.Copy,
                            scale=T["s2p"][:], bias=-2.0 * atau,
                        ))
                        L(f"E2nB.{b}", nc.vector.tensor_scalar(
                            T["E2n"][:, HALF:FW], T["ns3"][:, HALF:FW], T["s2p"][:],
                            -2.0 * atau, op0=ALU.mult, op1=ALU.add,
                        ))
                    else:
                        L(f"E2n.{b}", nc.scalar.activation(
                            T["E2n"][:], T["ns3"][:], AF.Copy,
                            scale=T["s2p"][:], bias=-2.0 * atau,
                        ))
                    T["psT"] = ps_T.tile([128, FW], f16, tag=("pT" if CFG.get("sharedT") else f"pT{b}"), name=f"pT{b}")
                    hpt = tc.high_priority(CFG.get("thprio")) if CFG.get("thprio") else contextlib.nullcontext()
                    with hpt:
                        for c in range(NF):
                            L(f"tr{c}.{b}", nc.tensor.transpose(
                                T["psT"][:, c * 128 : (c + 1) * 128],
                                T["E2n"][:, c * 128 : (c + 1) * 128],
                                I16[:],
                            ))
                    T["E2Tn"] = e2t_pool.tile([128, FW], f16, tag=f"e2t{b}", name=f"e2tn{b}")
                    hpe = tc.high_priority(CFG.get("ehprio")) if CFG.get("ehprio") else contextlib.nullcontext()
                    with hpe:
                        ev = CFG.get("evac", "dve")
                        if ev == "act":
                            L(f"evac.{b}", nc.scalar.activation(T["E2Tn"][:], T["psT"][:].bitcast(f16), AF.Copy))
                        elif ev == "split":
                            L(f"evacA.{b}", nc.scalar.activation(T["E2Tn"][:, 0:HALF], T["psT"][:, 0:HALF].bitcast(f16), AF.Copy))
                            L(f"evac.{b}", nc.vector.tensor_copy(T["E2Tn"][:, HALF:FW], T["psT"][:, HALF:FW]))
                        else:
                            L(f"evac.{b}", nc.vector.tensor_copy(T["E2Tn"][:], T["psT"][:]))
                    # q-chain for next iter
                    if EH_ROUTE:
                        T["h2"] = sc_pool.tile([128, FW], f16, tag=f"h2{b}", name=f"h2{b}")
                        L(f"h.{b}", nc.vector.tensor_tensor(T["h2"][:], T["hv"][:], T["E2n"][:], ALU.add))
                        qn_next[b] = q_pool.tile([128, FW], f16, tag=f"q{b}", name=f"qn{b}")
                        L(f"qn.{b}", nc.vector.tensor_scalar_max(qn_next[b][:], T["h2"][:], 0.0))
                        T["Ehn"] = sc_pool.tile([128, FW], f16, tag=f"eh{b}", name=f"ehn{b}")
                        L(f"Eh.{b}", nc.vector.tensor_scalar(
                            T["Ehn"][:], T["E2n"][:], -0.5, 0.0, op0=ALU.mult, op1=ALU.add
                        ))
                    else:
                        T["hA"] = sc_pool.tile([128, FW], f16, tag=f"hA{b}", name=f"hA{b}")
                        L(f"hA.{b}", nc.vector.scalar_tensor_tensor(
                            T["hA"][:], E2[b][:], -0.5, T["E2n"][:], op0=ALU.mult, op1=ALU.add
                        ))
                        T["h"] = sc_pool.tile([128, FW], f16, tag=f"h{b}", name=f"h{b}")
                        L(f"h.{b}", nc.vector.tensor_tensor(T["h"][:], T["hA"][:], T["w"][:], ALU.add))
                        qn_next[b] = q_pool.tile([128, FW], f16, tag=f"q{b}", name=f"qn{b}")
                        hpq = tc.high_priority(CFG.get("qhprio")) if CFG.get("qhprio") else contextlib.nullcontext()
                        with hpq:
                            L(f"qn.{b}", nc.vector.tensor_scalar_max(qn_next[b][:], T["h"][:], 0.0))
                elif s == 11:
                    if not last:
                        E2p[b] = E2[b]
                        E2[b] = T["E2n"]
                        E2Tp[b] = E2T[b]
                        E2T[b] = T["E2Tn"]
                        p_state[b] = T["p"]
                        if not PCN_ELIM:
                            pc[b] = T["pcn"]
                        if EH_ROUTE:
                            Eh[b] = T["Ehn"]

            total = N_ITERS * NSTG
            for gs in range(total + K_OFF):
                if gs < total:
                    emit(gs // NSTG, 0, gs % NSTG)
                g1 = gs - K_OFF
                if 0 <= g1 < total:
                    emit(g1 // NSTG, 1, g1 % NSTG)

    nc.finalize()
    return nc


def _get_nc(S: np.ndarray):
    key = (hash(S.tobytes()), tuple(sorted(CFG.items())))
    if key not in _BUILD_CACHE:
        L = _power_L(S)
        tau = 0.9 / L
        sigma = 0.9 / L
        _BUILD_CACHE[key] = (_build_nc(tau, sigma), tau, sigma)
    return _BUILD_CACHE[key]


def _make_in_maps(X, W1, b1, W2, b2, W3, b3, S, tau, sigma):
    f32 = np.float32
    alpha = np.float32(tau) * np.float32(sigma)
    atau = np.float32(alpha) * np.float32(tau)
    a16 = np.float16(alpha).astype(f32)
    Xflat = np.ascontiguousarray(X.reshape(B_FULL, N_COMBOS)).astype(f32)
    S = S.astype(f32)
    aST_full = (alpha * S.T).astype(f32)  # [512, 64]
    aST = np.ascontiguousarray(
        np.concatenate([aST_full[c * 128 : (c + 1) * 128, :] for c in range(NF)], axis=1)
    )
    STa_full = (-S.T).astype(np.float16)
    STa = np.ascontiguousarray(
        np.concatenate([STa_full[c * 128 : (c + 1) * 128, :] for c in range(NF)], axis=1)
    )
    STb_full = (0.5 * S.T).astype(np.float16)
    STb = np.ascontiguousarray(
        np.concatenate([STb_full[c * 128 : (c + 1) * 128, :] for c in range(NF)], axis=1)
    )
    AS16 = np.ascontiguousarray((a16 * S).astype(np.float16))
    I128 = np.eye(128, dtype=f32)
    sbias = np.ascontiguousarray((-atau * S.sum(axis=1)).astype(f32).reshape(N_COMBOS, 1))
    shared = {
        "w1": np.ascontiguousarray(W1.astype(f32)),
        "b1r": np.ascontiguousarray(b1.reshape(8, 128).T).astype(f32),
        "w2": np.ascontiguousarray(W2.astype(np.float16)),
        "b2r": np.ascontiguousarray(b2.reshape(8, 128).T).astype(f32),
        "w3": np.ascontiguousarray(W3.astype(np.float16)),
        "b3r": np.ascontiguousarray(b3.reshape(4, 128).T).astype(f32),
        "ast": aST,
        "sta16": STa,
        "stb16": STb,
        "as16": AS16,
        "nai16": np.ascontiguousarray((-a16 * I128).astype(np.float16)),
        "hi16": np.ascontiguousarray((0.5 * I128).astype(np.float16)),
        "i64_16": np.eye(N_COMBOS, dtype=np.float16),
        "i16": I128.astype(np.float16),
        "identr": I128,
        "sbias": sbias,
    }
    in_maps = []
    for c in range(N_CORES):
        xt = np.ascontiguousarray(Xflat[c * BC : (c + 1) * BC, :].T)
        in_maps.append({**shared, "xt": xt})
    return in_maps


def kernel(X, W1, b1, W2, b2, W3, b3, S, batch_size):
    from concourse.bass_utils import run_bass_kernel_spmd

    X = np.asarray(X)
    S = np.asarray(S)
    nc, tau, sigma = _get_nc(np.ascontiguousarray(S.astype(np.float32)))
    in_maps = _make_in_maps(
        X, np.asarray(W1), np.asarray(b1), np.asarray(W2), np.asarray(b2),
        np.asarray(W3), np.asarray(b3), S, tau, sigma,
    )
    res = run_bass_kernel_spmd(nc, in_maps, core_ids=list(range(N_CORES)))
    out = np.concatenate([res.results[c]["out"] for c in range(N_CORES)], axis=0)
    return out.astype(np.float32)


# revision 8
# speedup vs baseline: 1.0165x; 1.0012x over previous
"""Trainium2 Bass kernel v2 for nn_MatchNet (MLP forward + 60-iter batched PDHG).

Data-parallel over 8 NeuronCores: batch 2048 -> 256 rows/core (2 b-tiles of 128).

v2 reformulation vs baseline: state E2 := -2*alpha*(x - Z + tau) (fp16, N layout)
plus its transpose E2T kept as a state (2 rotating buffers each). The xbar
combination aeb = E2_k - 0.5*E2_{k-1} + atau is never materialized:
  - its transpose enters ps1 via two weight sets (STa = -S^T, STb = 0.5*S^T),
    with the atau*S@1 constant folded into cSZB.
  - the y2-chain uses hA = -0.5*E2_{k-1} + E2_k (one DVE STT op).
This removes the baseline's per-iteration DVE affine (658ns) + ns/s2p ops;
s2p = relu(2 - 2*nr) is computed on Act directly ([128,1] ops are ~free).

Per iteration (tile b):
  w   = q + naZ'         (Pool STT; naZ' = -alpha*Z + atau)
  hA  = -0.5*E2p + E2c   (DVE STT)
  h   = hA + w           (DVE TT)
  qn  = relu(h)          (DVE TSP)
  ps1 = I64@pc + sum_c STb_c@E2Tp_c + sum_c STa_c@E2Tc_c   (PE, all iter-start ready)
  p   = relu(ps1)        (cfg engine)
  pcn = p + cSZB'        (cfg engine)
  NS3 = hI@E2c + nAI@qn + p@AS16          (PE PSUM [128,512] = -alpha*d)
  n2  = sum(NS3^2)       (Act Square, out->PSUM junk, accum_out)
  nr  = rsqrt(n2/t2a2+eps) (Act)
  s2p = relu(2-2*nr)     (Act)
  E2n = s2p*NS3 - 2atau  (Act h1 + DVE h2, halves)
  psT = transpose(E2n)   (PE, per half)
  E2Tn = copy(psT)       (DVE, per half)
  out x = Z - E2/(2 alpha) - tau  (from final NS3: x = Z + (s2p/(-2 alpha))*NS3)
"""

import numpy as np

N_STRUCTS = 512
N_COMBOS = 64
HID = 1024
N_ITERS = 60
N_CORES = 8
B_FULL = 2048
BC = B_FULL // N_CORES  # 256 batch rows per core
NB = BC // 128  # 2 batch sub-tiles
NF = N_STRUCTS // 128  # 4 feature chunks

OPLOG = {}

CFG = {
    "p": "dve",
    "pc": "pool",
    "w": "pool",
    "e2n": "act",
    "halves": False,
    "koff": 6,
    "pcnelim": True,
    "ehroute": False,
    "phprio": 24,
    "warmact": True,
    "w1split": True,
    "out16": True,
}

_BUILD_CACHE = {}


def _power_L(S: np.ndarray) -> float:
    S = S.astype(np.float32)
    n = S.shape[1]
    v = np.full((n,), 1.0 / np.sqrt(n), np.float32)
    for _ in range(30):
        v2 = (S.T @ (S @ v) + v).astype(np.float32)
        v = (v2 / np.float32(np.linalg.norm(v2))).astype(np.float32)
    L = np.sqrt(np.vdot(v, (S.T @ (S @ v) + v).astype(np.float32)))
    return float(L)


def _build_nc(tau: float, sigma: float):
    import contextlib

    import concourse.bacc as bacc
    import concourse.mybir as mybir
    import concourse.tile as tile

    f32 = mybir.dt.float32
    f32r = mybir.dt.float32r
    f16 = mybir.dt.float16
    AF = mybir.ActivationFunctionType
    ALU = mybir.AluOpType
    alpha = float(np.float32(tau) * np.float32(sigma))
    atau = float(np.float32(alpha) * np.float32(tau))
    t2a2 = float((np.float32(tau) * np.float32(alpha)) ** 2)
    dsq_scale = float(1.0 / t2a2)

    nc = bacc.Bacc("TRN2", target_bir_lowering=False, debug=False)

    def L(label, inst):
        try:
            OPLOG[str(inst.ins.name)] = label
        except Exception:
            pass
        return inst

    def creg(v):
        key = (f32, v)
        if key not in nc.const_aps.aps:
            t = nc.alloc_sbuf_tensor(f"constx-{v}", [128, 1], f32)
            nc.gpsimd.memset(t.ap(), v)
            nc.const_aps.aps[key] = t.ap()
        return v

    creg(atau)
    creg(-2.0 * atau)
    creg(2.0)
    creg(0.0)
    creg(1e-6)

    # ---- DRAM I/O (per-core shapes) ----
    d_XT = nc.dram_tensor("xt", [N_COMBOS, BC], f32r, kind="ExternalInput")
    d_W1 = nc.dram_tensor("w1", [N_COMBOS, HID], f32r, kind="ExternalInput")
    d_b1 = nc.dram_tensor("b1r", [128, 8], f32, kind="ExternalInput")
    d_W2 = nc.dram_tensor("w2", [HID, HID], f16, kind="ExternalInput")
    d_b2 = nc.dram_tensor("b2r", [128, 8], f32, kind="ExternalInput")
    d_W3 = nc.dram_tensor("w3", [HID, N_STRUCTS], f16, kind="ExternalInput")
    d_b3 = nc.dram_tensor("b3r", [128, 4], f32, kind="ExternalInput")
    d_aST = nc.dram_tensor("ast", [128, NF * N_COMBOS], f32r, kind="ExternalInput")
    d_STa = nc.dram_tensor("sta16", [128, NF * N_COMBOS], f16, kind="ExternalInput")
    d_STb = nc.dram_tensor("stb16", [128, NF * N_COMBOS], f16, kind="ExternalInput")
    d_AS16 = nc.dram_tensor("as16", [N_COMBOS, N_STRUCTS], f16, kind="ExternalInput")
    d_nAI16 = nc.dram_tensor("nai16", [128, 128], f16, kind="ExternalInput")
    d_hI16 = nc.dram_tensor("hi16", [128, 128], f16, kind="ExternalInput")
    d_I64 = nc.dram_tensor("i64_16", [N_COMBOS, N_COMBOS], f16, kind="ExternalInput")
    d_I16 = nc.dram_tensor("i16", [128, 128], f16, kind="ExternalInput")
    d_Ir = nc.dram_tensor("identr", [128, 128], f32r, kind="ExternalInput")
    d_sb = nc.dram_tensor("sbias", [N_COMBOS, 1], f32, kind="ExternalInput")
    OUT16 = CFG.get("out16", False)
    d_out = nc.dram_tensor("out", [BC, N_STRUCTS], f16 if OUT16 else f32, kind="ExternalOutput")

    FW = N_STRUCTS  # 512

    with tile.TileContext(nc) as tc:
        stack = contextlib.ExitStack()
        with stack:
            cpool = stack.enter_context(tc.tile_pool(name="consts", bufs=1))

            def cload(dram, shape, tag, dt):
                t = cpool.tile(shape, dt, tag=tag, name=tag)
                nc.sync.dma_start(t[:], dram.ap())
                return t

            if CFG.get("warmact", False):
                # Dummy activation on a const tile: forces the ACT table load
                # (1283ns) to schedule at t~0 instead of before the first relu.
                wrm = cpool.tile([128, 1], f32, tag="warm", name="warm")
                nc.gpsimd.memset(wrm[:], 0.0)
                nc.scalar.activation(wrm[:], wrm[:], AF.Relu)
                nc.scalar.activation(wrm[:], wrm[:], AF.Square)
                nc.scalar.activation(wrm[:], wrm[:], AF.Abs_reciprocal_sqrt, scale=1.0, bias=1e-6)
            XT = cload(d_XT, [N_COMBOS, BC], "xt", f32r)
            if CFG.get("w1split", False):
                W1 = cpool.tile([N_COMBOS, HID], f32r, tag="w1", name="w1")
                nc.sync.dma_start(W1[:, 0:512], d_W1.ap()[:, 0:512])
                nc.sync.dma_start(W1[:, 512:HID], d_W1.ap()[:, 512:HID])
            else:
                W1 = cload(d_W1, [N_COMBOS, HID], "w1", f32r)
            b1r = cload(d_b1, [128, 8], "b1r", f32)

            # ---- MLP forward (float32r, T layout) ----
            zt = []  # Z^T tiles [128, BC] x4, f32r
            with (
                tc.tile_pool(name="mlp_sb", bufs=1) as mpool,
                tc.tile_pool(name="mlp_ps", bufs=1, space="PSUM") as mpsum,
            ):
                W2 = []
                for k in range(8):
                    t = mpool.tile([128, HID], f16, tag=f"w2_{k}", name=f"w2_{k}")
                    nc.sync.dma_start(t[:], d_W2.ap()[k * 128 : (k + 1) * 128, :])
                    W2.append(t)
                b2r = cload(d_b2, [128, 8], "b2r", f32)
                W3 = []
                for k in range(8):
                    t = mpool.tile([128, N_STRUCTS], f16, tag=f"w3_{k}", name=f"w3_{k}")
                    nc.sync.dma_start(t[:], d_W3.ap()[k * 128 : (k + 1) * 128, :])
                    W3.append(t)
                b3r = cload(d_b3, [128, 4], "b3r", f32)
                aST = cload(d_aST, [128, NF * N_COMBOS], "ast", f32r)
                Ir = cload(d_Ir, [128, 128], "identr", f32r)
                STa = cload(d_STa, [128, NF * N_COMBOS], "sta16", f16)
                STb = cload(d_STb, [128, NF * N_COMBOS], "stb16", f16)
                AS16 = cload(d_AS16, [N_COMBOS, N_STRUCTS], "as16", f16)
                nAI16 = cload(d_nAI16, [128, 128], "nai16", f16)
                hI16 = cload(d_hI16, [128, 128], "hi16", f16)
                I64 = cload(d_I64, [N_COMBOS, N_COMBOS], "i64_16", f16)
                I16 = cload(d_I16, [128, 128], "i16", f16)
                sbias = cload(d_sb, [N_COMBOS, 1], "sbias", f32)
                z1t = []
                for t in range(8):
                    ps = mpsum.tile([128, BC], f32, tag=f"zmm{t}", name=f"zmm{t}")
                    nc.tensor.matmul(
                        ps[:], W1[:, t * 128 : (t + 1) * 128], XT[:], start=True, stop=True
                    )
                    sb = mpool.tile([128, BC], f16, tag=f"z1_{t}", name=f"z1_{t}")
                    nc.scalar.activation(sb[:], ps[:], AF.Relu, bias=b1r[:, t : t + 1])
                    z1t.append(sb)
                zps2 = [
                    mpsum.tile([128, BC], f32, tag=f"zmm{t}", name=f"zmm2{t}")
                    for t in range(8)
                ]
                for k in range(8):
                    for t in range(8):
                        nc.tensor.matmul(
                            zps2[t][:],
                            W2[k][:, t * 128 : (t + 1) * 128],
                            z1t[k][:],
                            start=(k == 0),
                            stop=(k == 7),
                        )
                z2t = []
                for t in range(8):
                    sb = mpool.tile([128, BC], f16, tag=f"z2_{t}", name=f"z2_{t}")
                    nc.scalar.activation(sb[:], zps2[t][:], AF.Relu, bias=b2r[:, t : t + 1])
                    z2t.append(sb)
                zps3 = [
                    mpsum.tile([128, BC], f32, tag=f"zmm{c}", name=f"z3mm{c}")
                    for c in range(NF)
                ]
                for k in range(8):
                    for c in range(NF):
                        nc.tensor.matmul(
                            zps3[c][:],
                            W3[k][:, c * 128 : (c + 1) * 128],
                            z2t[k][:],
                            start=(k == 0),
                            stop=(k == 7),
                        )
                for c in range(NF):
                    sb = cpool.tile([128, BC], f32r, tag=f"zt_{c}", name=f"zt_{c}")
                    nc.scalar.activation(sb[:], zps3[c][:], AF.Relu, bias=b3r[:, c : c + 1])
                    zt.append(sb)

            # ---- PDHG setup ----
            spool = stack.enter_context(tc.tile_pool(name="setup", bufs=1))
            e2_pool = stack.enter_context(tc.tile_pool(name="e2p", bufs=3))
            e2t_pool = stack.enter_context(tc.tile_pool(name="e2tp", bufs=3))
            q_pool = stack.enter_context(tc.tile_pool(name="qp", bufs=3))
            p_pool = stack.enter_context(tc.tile_pool(name="pp", bufs=3))
            sc_pool = stack.enter_context(tc.tile_pool(name="scratch", bufs=3))
            with tc.tile_pool(name="pd_ps", bufs=1, space="PSUM") as ppool:
                # cSZB' = alpha*(S@Z^T - B^T) - atau*(S@1) 1^T   [64, BC] fp16
                ps = ppool.tile([N_COMBOS, BC], f32, tag="py1", name="py1")
                for c in range(NF):
                    nc.tensor.matmul(
                        ps[:], aST[:, c * 64 : (c + 1) * 64], zt[c][:],
                        start=(c == 0), stop=False,
                    )
                naI64 = spool.tile([N_COMBOS, N_COMBOS], f32r, tag="nai64", name="naI64")
                nc.scalar.activation(naI64[:], Ir[:64, :64].bitcast(f32), AF.Copy, scale=-alpha)
                nc.tensor.matmul(ps[:], naI64[:], XT[:], start=False, stop=True)
                cSZB0 = spool.tile([N_COMBOS, BC], f16, tag="cszb0", name="cSZB0")
                nc.scalar.activation(cSZB0[:], ps[:], AF.Copy)
                # cSZB' = cSZB + sbias (per-partition const, folds atau*S@1)
                cSZB = spool.tile([N_COMBOS, BC], f16, tag="cszb", name="cSZB")
                nc.vector.tensor_scalar(
                    cSZB[:], cSZB0[:], sbias[:], 0.0, op0=ALU.add, op1=ALU.add
                )

                # Z per-b in N layout (f32) via PE transposes
                Zf = []
                pszl = []
                for b in range(NB):
                    psz = ppool.tile([128, FW], f32r, tag=f"pz{b}", name=f"pz{b}")
                    pszl.append(psz)
                    for c in range(NF):
                        nc.tensor.transpose(
                            psz[:, c * 128 : (c + 1) * 128],
                            zt[c][:, b * 128 : (b + 1) * 128],
                            Ir[:],
                        )
                    zb = spool.tile([128, FW], f32, tag=f"zn{b}", name=f"zn{b}")
                    if CFG.get("zfdve"):
                        nc.vector.tensor_copy(zb[:], psz[:].bitcast(f32))
                    else:
                        nc.scalar.activation(zb[:], psz[:].bitcast(f32), AF.Copy)
                    Zf.append(zb)

                # state init
                naZ, E2, E2p, E2T, E2Tp, q, pc = [], [], [], [], [], [], []
                for b in range(NB):
                    t = spool.tile([128, FW], f16, tag=f"naz{b}", name=f"naz{b}")
                    zsrc = pszl[b][:].bitcast(f32) if CFG.get("pszinit") else Zf[b][:]
                    nc.scalar.activation(t[:], zsrc, AF.Copy, scale=-alpha, bias=atau)
                    naZ.append(t)
                    t = e2_pool.tile([128, FW], f16, tag=f"e2{b}", name=f"e2i{b}")
                    nc.vector.tensor_scalar(t[:], zsrc, 2.0 * alpha, -2.0 * atau, op0=ALU.mult, op1=ALU.add)
                    E2.append(t)
                    E2p.append(t)  # E2_{-1} = E2_0
                    # E2T_0 via PE transpose + evac
                    pst = ppool.tile([128, FW], f16, tag=f"pTi{b}", name=f"pTi{b}")
                    for c in range(NF):
                        nc.tensor.transpose(
                            pst[:, c * 128 : (c + 1) * 128],
                            t[:, c * 128 : (c + 1) * 128],
                            I16[:],
                        )
                    tt_ = e2t_pool.tile([128, FW], f16, tag=f"e2t{b}", name=f"e2ti{b}")
                    nc.vector.tensor_copy(tt_[:], pst[:])
                    E2T.append(tt_)
                    E2Tp.append(tt_)
                    tq = q_pool.tile([128, FW], f16, tag=f"q{b}", name=f"qi{b}")
                    nc.gpsimd.memset(tq[:], 0.0)
                    q.append(tq)
                    if not CFG.get("pcnelim", False):
                        tp_ = p_pool.tile([N_COMBOS, 128], f16, tag=f"pc{b}", name=f"pci{b}")
                        nc.vector.tensor_copy(tp_[:], cSZB[:, b * 128 : (b + 1) * 128])
                        pc.append(tp_)

            ps_T = stack.enter_context(tc.tile_pool(name="ps_T", bufs=(2 if CFG.get("sharedT") else CFG.get("bufsT", 1)), space="PSUM"))
            ps_y1 = stack.enter_context(tc.tile_pool(name="ps_y1", bufs=CFG.get("bufsY", 1), space="PSUM"))
            ps_3 = stack.enter_context(tc.tile_pool(name="ps_3", bufs=(2 if CFG.get("shared3") else 1), space="PSUM"))
            ps_j = stack.enter_context(tc.tile_pool(name="ps_j", bufs=1, space="PSUM"))

            # ---- iteration emission (v1 shape: trans+evac right after E2n) ----
            NSTG = 12
            K_OFF = CFG.get("koff", 6)
            temps = [dict(), dict()]
            HALF = FW // 2
            PCN_ELIM = CFG.get("pcnelim", False)
            EH_ROUTE = CFG.get("ehroute", False)

            # Analytic init: h_0 = w_0 + aeb_0 = (atau - aZ) + aZ ... = 0 exactly,
            # so qn_0 = 0; and Eh_0 = -0.5*E2_0 = -aZ + atau = naZ (alias).
            Eh = list(naZ)
            qn_next = []
            for b in range(NB):
                t = q_pool.tile([128, FW], f16, tag=f"q{b}", name=f"qn0{b}")
                nc.gpsimd.memset(t[:], 0.0)
                qn_next.append(t)
            qn = [None, None]
            p_state = []
            for b in range(NB):
                t = p_pool.tile([N_COMBOS, 128], f16, tag=f"p{b}", name=f"pz{b}")
                nc.gpsimd.memset(t[:], 0.0)
                p_state.append(t)

            def emit(it, b, s):
                T = temps[b]
                last = it == N_ITERS - 1
                if s == 0:
                    qn[b] = qn_next[b]
                    # w_{k+1} = qn_k + naZ (Pool) -- for NEXT iter's h
                    if not last:
                        T["w"] = sc_pool.tile([128, FW], f16, tag=f"w{b}", name=f"w{b}")
                        if CFG["w"] == "pool":
                            L(f"w.{b}", nc.gpsimd.tensor_tensor(
                                T["w"][:], qn[b][:], naZ[b][:], ALU.add
                            ))
                        else:
                            L(f"w.{b}", nc.vector.tensor_tensor(T["w"][:], qn[b][:], naZ[b][:], ALU.add))
                elif s == 1:
                    if EH_ROUTE and not last:
                        T["hv"] = sc_pool.tile([128, FW], f16, tag=f"hv{b}", name=f"hv{b}")
                        L(f"hv.{b}", nc.vector.tensor_tensor(T["hv"][:], T["w"][:], Eh[b][:], ALU.add))
                elif s == 2:
                    if CFG.get("earlyps1"):
                        T["ps1"] = ps_y1.tile([N_COMBOS, 128], f32, tag=f"py{b}", name=f"py{b}")
                        L(f"ps1c.{b}", nc.tensor.matmul(
                            T["ps1"][:], I64[:], cSZB[:, b * 128 : (b + 1) * 128],
                            start=True, stop=False,
                        ))
                        L(f"ps1p.{b}", nc.tensor.matmul(
                            T["ps1"][:], I64[:], p_state[b][:], start=False, stop=False,
                        ))
                        for c in range(NF):
                            L(f"ps1b{c}.{b}", nc.tensor.matmul(
                                T["ps1"][:],
                                STb[:, c * 64 : (c + 1) * 64],
                                E2Tp[b][:, c * 128 : (c + 1) * 128],
                                start=False, stop=False,
                            ))
                elif s == 3:
                    pass
                elif s == 4:
                    if CFG.get("earlyps1"):
                        for c in range(NF):
                            L(f"ps1a{c}.{b}", nc.tensor.matmul(
                                T["ps1"][:],
                                STa[:, c * 64 : (c + 1) * 64],
                                E2T[b][:, c * 128 : (c + 1) * 128],
                                start=False, stop=(c == NF - 1),
                            ))
                        return
                    T["ps1"] = ps_y1.tile([N_COMBOS, 128], f32, tag=f"py{b}", name=f"py{b}")
                    if PCN_ELIM:
                        L(f"ps1c.{b}", nc.tensor.matmul(
                            T["ps1"][:], I64[:], cSZB[:, b * 128 : (b + 1) * 128],
                            start=True, stop=False,
                        ))
                        L(f"ps1p.{b}", nc.tensor.matmul(
                            T["ps1"][:], I64[:], p_state[b][:], start=False, stop=False,
                        ))
                    else:
                        L(f"ps1c.{b}", nc.tensor.matmul(
                            T["ps1"][:], I64[:], pc[b][:], start=True, stop=False,
                        ))
                    for c in range(NF):
                        L(f"ps1b{c}.{b}", nc.tensor.matmul(
                            T["ps1"][:],
                            STb[:, c * 64 : (c + 1) * 64],
                            E2Tp[b][:, c * 128 : (c + 1) * 128],
                            start=False, stop=False,
                        ))
                    for c in range(NF):
                        L(f"ps1a{c}.{b}", nc.tensor.matmul(
                            T["ps1"][:],
                            STa[:, c * 64 : (c + 1) * 64],
                            E2T[b][:, c * 128 : (c + 1) * 128],
                            start=False, stop=(c == NF - 1),
                        ))
                elif s == 5:
                    T["p"] = p_pool.tile([N_COMBOS, 128], f16, tag=f"p{b}", name=f"p{b}")
                    hp = tc.high_priority(CFG.get("phprio")) if CFG.get("phprio") else contextlib.nullcontext()
                    with hp:
                        if CFG["p"] == "act":
                            L(f"p.{b}", nc.scalar.activation(T["p"][:], T["ps1"][:], AF.Relu))
                        else:
                            L(f"p.{b}", nc.vector.tensor_scalar_max(T["p"][:], T["ps1"][:], 0.0))
                elif s == 6:
                    if not PCN_ELIM and not last:
                        T["pcn"] = p_pool.tile([N_COMBOS, 128], f16, tag=f"pc{b}", name=f"pc{b}")
                        if CFG["pc"] == "pool":
                            L(f"pcn.{b}", nc.gpsimd.tensor_tensor(
                                T["pcn"][:], T["p"][:], cSZB[:, b * 128 : (b + 1) * 128], ALU.add
                            ))
                        else:
                            L(f"pcn.{b}", nc.vector.tensor_tensor(
                                T["pcn"][:], T["p"][:], cSZB[:, b * 128 : (b + 1) * 128], ALU.add
                            ))
                elif s == 7:
                    T["ns3"] = ps_3.tile([128, FW], f32, tag=("p3" if CFG.get("shared3") else f"p3{b}"), name=f"p3{b}")
                    L(f"mm1.{b}", nc.tensor.matmul(T["ns3"][:], hI16[:], E2[b][:], start=True, stop=False))
                    L(f"mm2.{b}", nc.tensor.matmul(T["ns3"][:], nAI16[:], qn[b][:], start=False, stop=False))
                    hpc = tc.high_priority(CFG.get("chprio")) if CFG.get("chprio") else contextlib.nullcontext()
                    with hpc:
                        L(f"mmC.{b}", nc.tensor.matmul(T["ns3"][:], T["p"][:], AS16[:], start=False, stop=True))
                elif s == 8:
                    T["n2"] = sc_pool.tile([128, 1], f32, tag=f"n2{b}", name=f"n2{b}")
                    if CFG.get("jsbuf"):
                        dsqj = sc_pool.tile([128, FW], f32, tag="dsqj", name=f"dsqj{b}")
                    else:
                        dsqj = ps_j.tile([128, FW], f32, tag="dsqj", name=f"dsqj{b}")
                    L(f"dsq.{b}", nc.scalar.activation(dsqj[:], T["ns3"][:], AF.Square, accum_out=T["n2"][:]))
                elif s == 9:
                    T["nr"] = sc_pool.tile([128, 1], f32, tag=f"nr{b}", name=f"nr{b}")
                    L(f"nr.{b}", nc.scalar.activation(
                        T["nr"][:], T["n2"][:], AF.Abs_reciprocal_sqrt, scale=dsq_scale, bias=1e-6
                    ))
                    T["s2p"] = sc_pool.tile([128, 1], f32, tag=f"s2p{b}", name=f"s2p{b}")
                    L(f"s2p.{b}", nc.scalar.activation(T["s2p"][:], T["nr"][:], AF.Relu, scale=-2.0, bias=2.0))
                elif s == 10:
                    if last:
                        nsa = sc_pool.tile([128, 1], f32, tag=f"nsa{b}", name=f"nsa{b}")
                        nc.vector.tensor_scalar(
                            nsa[:], T["s2p"][:], -0.5 / alpha, 0.0, op0=ALU.mult, op1=ALU.add
                        )
                        xout = sc_pool.tile([128, FW], f16 if OUT16 else f32, tag=f"xo{b}", name=f"xo{b}")
                        if CFG.get("xsplit", False):
                            for hh in range(2):
                                lo, hi = hh * HALF, (hh + 1) * HALF
                                nc.vector.affine_then_add(
                                    xout[:, lo:hi], T["ns3"][:, lo:hi], Zf[b][:, lo:hi],
                                    scale=nsa[:], bias=0.0
                                )
                                nc.sync.dma_start(
                                    d_out.ap()[b * 128 : (b + 1) * 128, lo:hi], xout[:, lo:hi]
                                )
                        else:
                            nc.vector.affine_then_add(
                                xout[:], T["ns3"][:], Zf[b][:], scale=nsa[:], bias=0.0
                            )
                            nc.sync.dma_start(d_out.ap()[b * 128 : (b + 1) * 128, :], xout[:])
                        return
                    T["E2n"] = e2_pool.tile([128, FW], f16, tag=f"e2{b}", name=f"e2n{b}")
                    if CFG["e2n"] == "split":
                        L(f"E2nA.{b}", nc.scalar.activation(
                            T["E2n"][:, 0:HALF], T["ns3"][:, 0:HALF], AF.Copy,
                            scale=T["s2p"][:], bias=-2.0 * atau,
                        ))
                        L(f"E2nB.{b}", nc.vector.tensor_scalar(
                            T["E2n"][:, HALF:FW], T["ns3"][:, HALF:FW], T["s2p"][:],
                            -2.0 * atau, op0=ALU.mult, op1=ALU.add,
                        ))
                    else:
                        L(f"E2n.{b}", nc.scalar.activation(
                            T["E2n"][:], T["ns3"][:], AF.Copy,
                            scale=T["s2p"][:], bias=-2.0 * atau,
                        ))
                    T["psT"] = ps_T.tile([128, FW], f16, tag=("pT" if CFG.get("sharedT") else f"pT{b}"), name=f"pT{b}")
                    hpt = tc.high_priority(CFG.get("thprio")) if CFG.get("thprio") else contextlib.nullcontext()
                    with hpt:
                        for c in range(NF):
                            L(f"tr{c}.{b}", nc.tensor.transpose(
                                T["psT"][:, c * 128 : (c + 1) * 128],
                                T["E2n"][:, c * 128 : (c + 1) * 128],
                                I16[:],
                            ))
                    T["E2Tn"] = e2t_pool.tile([128, FW], f16, tag=f"e2t{b}", name=f"e2tn{b}")
                    hpe = tc.high_priority(CFG.get("ehprio")) if CFG.get("ehprio") else contextlib.nullcontext()
                    with hpe:
                        ev = CFG.get("evac", "dve")
                        if ev == "act":
                            L(f"evac.{b}", nc.scalar.activation(T["E2Tn"][:], T["psT"][:].bitcast(f16), AF.Copy))
                        elif ev == "split":
                            L(f"evacA.{b}", nc.scalar.activation(T["E2Tn"][:, 0:HALF], T["psT"][:, 0:HALF].bitcast(f16), AF.Copy))
                            L(f"evac.{b}", nc.vector.tensor_copy(T["E2Tn"][:, HALF:FW], T["psT"][:, HALF:FW]))
                        else:
                            L(f"evac.{b}", nc.vector.tensor_copy(T["E2Tn"][:], T["psT"][:]))
                    # q-chain for next iter
                    if EH_ROUTE:
                        T["h2"] = sc_pool.tile([128, FW], f16, tag=f"h2{b}", name=f"h2{b}")
                        L(f"h.{b}", nc.vector.tensor_tensor(T["h2"][:], T["hv"][:], T["E2n"][:], ALU.add))
                        qn_next[b] = q_pool.tile([128, FW], f16, tag=f"q{b}", name=f"qn{b}")
                        L(f"qn.{b}", nc.vector.tensor_scalar_max(qn_next[b][:], T["h2"][:], 0.0))
                        T["Ehn"] = sc_pool.tile([128, FW], f16, tag=f"eh{b}", name=f"ehn{b}")
                        L(f"Eh.{b}", nc.vector.tensor_scalar(
                            T["Ehn"][:], T["E2n"][:], -0.5, 0.0, op0=ALU.mult, op1=ALU.add
                        ))
                    else:
                        T["hA"] = sc_pool.tile([128, FW], f16, tag=f"hA{b}", name=f"hA{b}")
                        L(f"hA.{b}", nc.vector.scalar_tensor_tensor(
                            T["hA"][:], E2[b][:], -0.5, T["E2n"][:], op0=ALU.mult, op1=ALU.add
                        ))
                        T["h"] = sc_pool.tile([128, FW], f16, tag=f"h{b}", name=f"h{b}")
                        L(f"h.{b}", nc.vector.tensor_tensor(T["h"][:], T["hA"][:], T["w"][:], ALU.add))
                        qn_next[b] = q_pool.tile([128, FW], f16, tag=f"q{b}", name=f"qn{b}")
                        hpq = tc.high_priority(CFG.get("qhprio")) if CFG.get("qhprio") else contextlib.nullcontext()
                        with hpq:
                            L(f"qn.{b}", nc.vector.tensor_scalar_max(qn_next[b][:], T["h"][:], 0.0))
                elif s == 11:
                    if not last:
                        E2p[b] = E2[b]
                        E2[b] = T["E2n"]
                        E2Tp[b] = E2T[b]
                        E2T[b] = T["E2Tn"]
                        p_state[b] = T["p"]
                        if not PCN_ELIM:
                            pc[b] = T["pcn"]
                        if EH_ROUTE:
                            Eh[b] = T["Ehn"]

            total = N_ITERS * NSTG
            for gs in range(total + K_OFF):
                if gs < total:
                    emit(gs // NSTG, 0, gs % NSTG)
                g1 = gs - K_OFF
                if 0 <= g1 < total:
                    emit(g1 // NSTG, 1, g1 % NSTG)

    nc.finalize()
    return nc


def _get_nc(S: np.ndarray):
    key = (hash(S.tobytes()), tuple(sorted(CFG.items())))
    if key not in _BUILD_CACHE:
        L = _power_L(S)
        tau = 0.9 / L
        sigma = 0.9 / L
        _BUILD_CACHE[key] = (_build_nc(tau, sigma), tau, sigma)
    return _BUILD_CACHE[key]


def _make_in_maps(X, W1, b1, W2, b2, W3, b3, S, tau, sigma):
    f32 = np.float32
    alpha = np.float32(tau) * np.float32(sigma)
    atau = np.float32(alpha) * np.float32(tau)
    a16 = np.float16(alpha).astype(f32)
    Xflat = np.ascontiguousarray(X.reshape(B_FULL, N_COMBOS)).astype(f32)
    S = S.astype(f32)
    aST_full = (alpha * S.T).astype(f32)  # [512, 64]
    aST = np.ascontiguousarray(
        np.concatenate([aST_full[c * 128 : (c + 1) * 128, :] for c in range(NF)], axis=1)
    )
    STa_full = (-S.T).astype(np.float16)
    STa = np.ascontiguousarray(
        np.concatenate([STa_full[c * 128 : (c + 1) * 128, :] for c in range(NF)], axis=1)
    )
    STb_full = (0.5 * S.T).astype(np.float16)
    STb = np.ascontiguousarray(
        np.concatenate([STb_full[c * 128 : (c + 1) * 128, :] for c in range(NF)], axis=1)
    )
    AS16 = np.ascontiguousarray((a16 * S).astype(np.float16))
    I128 = np.eye(128, dtype=f32)
    sbias = np.ascontiguousarray((-atau * S.sum(axis=1)).astype(f32).reshape(N_COMBOS, 1))
    shared = {
        "w1": np.ascontiguousarray(W1.astype(f32)),
        "b1r": np.ascontiguousarray(b1.reshape(8, 128).T).astype(f32),
        "w2": np.ascontiguousarray(W2.astype(np.float16)),
        "b2r": np.ascontiguousarray(b2.reshape(8, 128).T).astype(f32),
        "w3": np.ascontiguousarray(W3.astype(np.float16)),
        "b3r": np.ascontiguousarray(b3.reshape(4, 128).T).astype(f32),
        "ast": aST,
        "sta16": STa,
        "stb16": STb,
        "as16": AS16,
        "nai16": np.ascontiguousarray((-a16 * I128).astype(np.float16)),
        "hi16": np.ascontiguousarray((0.5 * I128).astype(np.float16)),
        "i64_16": np.eye(N_COMBOS, dtype=np.float16),
        "i16": I128.astype(np.float16),
        "identr": I128,
        "sbias": sbias,
    }
    in_maps = []
    for c in range(N_CORES):
        xt = np.ascontiguousarray(Xflat[c * BC : (c + 1) * BC, :].T)
        in_maps.append({**shared, "xt": xt})
    return in_maps


def kernel(X, W1, b1, W2, b2, W3, b3, S, batch_size):
    from concourse.bass_utils import run_bass_kernel_spmd

    X = np.asarray(X)
    S = np.asarray(S)
    nc, tau, sigma = _get_nc(np.ascontiguousarray(S.astype(np.float32)))
    in_maps = _make_in_maps(
        X, np.asarray(W1), np.asarray(b1), np.asarray(W2), np.asarray(b2),
        np.asarray(W3), np.asarray(b3), S, tau, sigma,
    )
    res = run_bass_kernel_spmd(nc, in_maps, core_ids=list(range(N_CORES)))
    out = np.concatenate([res.results[c]["out"] for c in range(N_CORES)], axis=0)
    return out.astype(np.float32)
